# revision 1
# baseline (speedup 1.0000x reference)
"""HAN layer (3-metapath GCN mean) Trainium2 Bass kernel, 8-core SPMD.

Strategy (dst-sharded pull):
  - nodes range-sharded across 8 cores (6250 each); every core computes the
    full x_m = (h * rsqrt(deg_out_m)) @ W_m table (bf16 matmul, fp32 result)
    into two DRAM tables per metapath of <32768 rows each (int16 index limit
    of dma_gather), each with 128 trailing zero rows for padding slots.
    The h * rsqrt(deg_out_m) pre-scaling is folded on the host into three
    bf16 hsT tables so phase 1 is DMA + PE + one ACT copy per (slab, m).
  - per (core, metapath, chunk): in-edges of owned nodes are laid out by the
    host into a gather slot schedule: nodes sorted by chunk-degree descending,
    blocks of 128 nodes, per-block fixed column count T[b] (elementwise max
    over the 8 cores so the program is identical across cores).  4096-idx
    dma_gather calls rotate over the 4 SWDGE queues (descriptor-generation
    rate is the wall: ~2.2ns/descriptor with all queues busy); VectorE
    tensor_reduce sums each block's columns; raw block sums are
    dma_scatter_add-ed (un-permuting) into a zeroed per-metapath DRAM
    aggregate.  A final per-metapath pass applies rsqrt(deg_in)/3, bias/3,
    relu (DVE scalar_tensor_tensor + ACT relu) and accumulates the mean.
  - host concatenates the 8 core outputs.
"""

import os

import numpy as np
import ml_dtypes

import concourse.bass as bass
import concourse.tile as tile
from concourse import bacc, mybir
from concourse.bass_utils import run_bass_kernel_spmd

F_IN, F_OUT, NMP = 128, 64, 3
_KIND = "ExternalOutput" if os.environ.get("KDEBUG") else "Internal"
GROUP_MAX_COLS = 32  # max supertile columns per dma_gather call (4096 idxs)


def _wrap16(flat):
    """slot i -> (partition i%16, free i//16), replicated to 128 partitions."""
    a = flat.astype(np.int16).reshape(-1, 16).T.copy()  # [16, S/16]
    return np.tile(a, (8, 1))


class _NS:
    pass


def _make_plan(N, ncores):
    p = _NS()
    p.N, p.ncores = N, ncores
    p.npc = N // ncores
    p.NBP = (p.npc + 127) // 128
    p.npc_pad = p.NBP * 128
    p.NT = (N + 511) // 512 * 4
    p.N_pad = p.NT * 128
    p.tilesA = (p.NT + 1) // 2
    p.tilesB = p.NT - p.tilesA
    p.CHN = p.tilesA * 128
    p.rowsA = p.tilesA * 128 + 128
    p.rowsB = p.tilesB * 128 + 128
    p.zeroA = p.tilesA * 128
    p.zeroB = p.tilesB * 128
    assert p.rowsA < 32768 and p.rowsB < 32768
    return p


def _build_stream(plan, d_sel, li_sel, order, Ts, zero_base):
    TOT = int(Ts.sum())
    fill = (zero_base + (np.arange(TOT * 128) % 128)).astype(np.int16)
    if TOT == 0 or len(d_sel) == 0:
        return fill
    B = np.zeros(plan.NBP, np.int64)
    B[1:] = np.cumsum(Ts)[:-1]
    rank = np.empty(plan.npc, np.int64)
    rank[order] = np.arange(plan.npc)
    r_e = rank[d_sel]
    o = np.argsort(r_e, kind="stable")
    r_s = r_e[o]
    li = li_sel[o]
    starts = np.searchsorted(r_s, np.arange(plan.npc))
    k = np.arange(len(r_s)) - starts[r_s]
    blk = r_s // 128
    col = B[blk] + k
    assert (k < Ts[blk]).all()
    fill[col * 128 + (r_s % 128)] = li.astype(np.int16)
    return fill


def _groups(Ts):
    out, b, col, NB = [], 0, 0, len(Ts)
    while b < NB:
        if Ts[b] == 0:
            b += 1
            continue
        b_lo, col_lo, ncols = b, col, 0
        while b < NB and Ts[b] > 0 and (ncols == 0 or ncols + Ts[b] <= GROUP_MAX_COLS):
            ncols += int(Ts[b])
            col += int(Ts[b])
            b += 1
        out.append((b_lo, b, col_lo, ncols))
    return out


def _prepare(plan, h, srcs, dsts, Ws, bs):
    N, npc = plan.N, plan.npc

    rs_out, rs_in = [], []
    for m in range(NMP):
        do = np.clip(np.bincount(srcs[m], minlength=N), 1, None).astype(np.float64)
        di = np.clip(np.bincount(dsts[m], minlength=N), 1, None).astype(np.float64)
        rs_out.append((1.0 / np.sqrt(do)).astype(np.float32))
        rs_in.append((1.0 / np.sqrt(di)).astype(np.float32))

    # pre-scaled transposed h tables, one per metapath (bf16)
    hsT = []
    for m in range(NMP):
        hp = np.zeros((plan.N_pad, F_IN), np.float32)
        hp[:N] = h * rs_out[m][:, None]
        hsT.append(np.ascontiguousarray(hp.T).astype(ml_dtypes.bfloat16))

    wall = np.concatenate(Ws, axis=1).astype(ml_dtypes.bfloat16)
    ball3 = np.concatenate(
        [np.tile(b[None, :] / 3.0, (128, 1)) for b in bs], axis=1
    ).astype(np.float32)

    info = {}
    T0s = [np.zeros(plan.NBP, np.int64) for _ in range(NMP)]
    T1s = [np.zeros(plan.NBP, np.int64) for _ in range(NMP)]
    pad = plan.npc_pad - npc
    z = np.zeros(pad, np.int64)
    for c in range(plan.ncores):
        lo = c * npc
        for m in range(NMP):
            sel = (dsts[m] >= lo) & (dsts[m] < lo + npc)
            s = srcs[m][sel]
            d = dsts[m][sel] - lo
            in0 = s < plan.CHN
            c0 = np.bincount(d[in0], minlength=npc)
            c1 = np.bincount(d[~in0], minlength=npc)
            o0 = np.argsort(-c0, kind="stable")
            o1 = np.argsort(-c1, kind="stable")
            info[(c, m)] = (s, d, in0, o0, o1)
            T0s[m] = np.maximum(
                T0s[m], np.concatenate([c0[o0], z]).reshape(plan.NBP, 128).max(1)
            )
            T1s[m] = np.maximum(
                T1s[m], np.concatenate([c1[o1], z]).reshape(plan.NBP, 128).max(1)
            )

    cfg = _NS()
    cfg.plan = plan
    cfg.T0s, cfg.T1s = T0s, T1s
    cfg.g0 = [_groups(T0s[m]) for m in range(NMP)]
    cfg.g1 = [_groups(T1s[m]) for m in range(NMP)]
    cfg.TOT0 = [int(T0s[m].sum()) for m in range(NMP)]
    cfg.TOT1 = [int(T1s[m].sum()) for m in range(NMP)]

    in_maps = []
    for c in range(plan.ncores):
        im = {"wall": wall, "ball3": ball3}
        for m in range(NMP):
            im[f"hs{m}"] = hsT[m]
        lo = c * npc
        for m in range(NMP):
            s, d, in0, o0, o1 = info[(c, m)]
            sA = s[in0]
            liA = (sA % 128) * plan.tilesA + sA // 128
            sB = s[~in0] - plan.CHN
            liB = (sB % 128) * plan.tilesB + sB // 128
            st0 = _build_stream(plan, d[in0], liA, o0, T0s[m], plan.zeroA)
            st1 = _build_stream(plan, d[~in0], liB, o1, T1s[m], plan.zeroB)
            im[f"g0_{m}"] = (
                _wrap16(st0) if cfg.TOT0[m] else np.zeros((128, 8), np.int16)
            )
            im[f"g1_{m}"] = (
                _wrap16(st1) if cfg.TOT1[m] else np.zeros((128, 8), np.int16)
            )
            for ch, o in ((0, o0), (1, o1)):
                si = np.full(plan.npc_pad, -1, np.int64)
                si[:npc] = o
                im[f"si{ch}_{m}"] = _wrap16(si)
            full = np.zeros(plan.npc_pad, np.float32)
            full[:npc] = rs_in[m][lo : lo + npc] / 3.0
            im[f"ri_{m}"] = full.reshape(plan.NBP, 128).T.copy()
        in_maps.append(im)
    return cfg, in_maps


def _build_kernel(cfg):
    plan = cfg.plan
    nc = bacc.Bacc(
        "TRN2",
        target_bir_lowering=False,
        debug=False,
        num_devices=plan.ncores,
        num_swdge_queues=4,
    )
    dt = mybir.dt
    hs_d = [
        nc.dram_tensor(f"hs{m}", (128, plan.N_pad), dt.bfloat16, kind="ExternalInput").ap()
        for m in range(NMP)
    ]
    wall_d = nc.dram_tensor("wall", (128, F_OUT * NMP), dt.bfloat16, kind="ExternalInput").ap()
    ball3_d = nc.dram_tensor("ball3", (128, F_OUT * NMP), dt.float32, kind="ExternalInput").ap()
    g_d, si_d, ri_d, xA, xB, agg = {}, {}, {}, {}, {}, {}
    for m in range(NMP):
        w0 = max(cfg.TOT0[m] * 8, 8)
        w1 = max(cfg.TOT1[m] * 8, 8)
        g_d[(0, m)] = nc.dram_tensor(f"g0_{m}", (128, w0), dt.int16, kind="ExternalInput").ap()
        g_d[(1, m)] = nc.dram_tensor(f"g1_{m}", (128, w1), dt.int16, kind="ExternalInput").ap()
        si_d[(0, m)] = nc.dram_tensor(f"si0_{m}", (128, plan.npc_pad // 16), dt.int16, kind="ExternalInput").ap()
        si_d[(1, m)] = nc.dram_tensor(f"si1_{m}", (128, plan.npc_pad // 16), dt.int16, kind="ExternalInput").ap()
        ri_d[m] = nc.dram_tensor(f"ri_{m}", (128, plan.NBP), dt.float32, kind="ExternalInput").ap()
        xA[m] = nc.dram_tensor(f"xA_{m}", (plan.rowsA, F_OUT), dt.float32, kind=_KIND).ap()
        xB[m] = nc.dram_tensor(f"xB_{m}", (plan.rowsB, F_OUT), dt.float32, kind=_KIND).ap()
        agg[m] = nc.dram_tensor(f"agg_{m}", (plan.npc_pad, F_OUT), dt.float32, kind=_KIND).ap()
    out_d = nc.dram_tensor("out", (plan.npc_pad, F_OUT), dt.float32, kind="ExternalOutput").ap()

    with tile.TileContext(nc) as tc:
        with tc.tile_pool(name="const", bufs=1) as constp, \
             tc.tile_pool(name="ht", bufs=6) as htp, \
             tc.tile_pool(name="ps", bufs=4, space="PSUM") as psp, \
             tc.tile_pool(name="xs", bufs=4) as xsp, \
             tc.tile_pool(name="gst", bufs=2) as gstp, \
             tc.tile_pool(name="gath", bufs=5) as gp, \
             tc.tile_pool(name="accs", bufs=2) as accp, \
             tc.tile_pool(name="fin", bufs=1) as finp, \
             tc.tile_pool(name="mean", bufs=1) as meanp:

            wall_t = constp.tile([128, F_OUT * NMP], dt.bfloat16)
            nc.sync.dma_start(wall_t[:], wall_d[:])
            ball3_t = constp.tile([128, F_OUT * NMP], dt.float32)
            nc.sync.dma_start(ball3_t[:], ball3_d[:])
            ri_t, si_t = {}, {}
            for m in range(NMP):
                ri_t[m] = constp.tile([128, plan.NBP], dt.float32, name=f"ri_t{m}")
                nc.sync.dma_start(ri_t[m][:], ri_d[m][:])
                for ch in (0, 1):
                    si_t[(ch, m)] = constp.tile(
                        [128, plan.npc_pad // 16], dt.int16, name=f"si_t{ch}_{m}"
                    )
                    nc.sync.dma_start(si_t[(ch, m)][:], si_d[(ch, m)][:])

            zt = constp.tile([128, F_OUT], dt.float32)
            nc.vector.memset(zt[:], 0.0)
            for m in range(NMP):
                nc.sync.dma_start(xA[m][plan.zeroA : plan.zeroA + 128, :], zt[:])
                nc.sync.dma_start(xB[m][plan.zeroB : plan.zeroB + 128, :], zt[:])
            mean_t = meanp.tile([128, plan.NBP, F_OUT], dt.float32)
            nc.vector.memset(mean_t[:], 0.0)
            for m in range(NMP):
                nc.sync.dma_start(
                    agg[m][:].rearrange("n f -> (n f)").rearrange(
                        "(p x) -> p x", p=128
                    ),
                    mean_t[:].rearrange("p t f -> p (t f)"),
                )

            # idx mega-stream preload for the first (m=0, chunk A) stream, so
            # its gathers can start as soon as xA_0 is written.
            gst_t = {}

            def load_stream(ch, m):
                tot = cfg.TOT0[m] if ch == 0 else cfg.TOT1[m]
                w = max(tot * 8, 8)
                t = gstp.tile([128, w], dt.int16, name="gst")
                nc.sync.dma_start(t[:], g_d[(ch, m)][:, :w])
                gst_t[(ch, m)] = t

            load_stream(0, 0)

            # ---- phase 1: x_m tables; all chunk-A tables first (m order), so
            # gathers (which consume chunk A of m=0 first) start early.
            SLAB = 4
            nslabA = plan.tilesA // SLAB
            slab_order = []
            for m in range(NMP):
                for slab in range(plan.N_pad // 512):
                    slab_order.append((m, slab))
            assert plan.tilesA % SLAB == 0 and plan.tilesB % SLAB == 0
            for m, slab in slab_order:
                t0 = slab * SLAB
                hts = htp.tile([128, 512], dt.bfloat16, name="hts")
                nc.sync.dma_start(hts[:], hs_d[m][:, slab * 512 : (slab + 1) * 512])
                ps = psp.tile([128, SLAB, F_OUT], dt.float32, space="PSUM")
                for j in range(SLAB):
                    nc.tensor.matmul(
                        ps[:, j, :],
                        lhsT=hts[:, j * 128 : (j + 1) * 128],
                        rhs=wall_t[:, m * F_OUT : (m + 1) * F_OUT],
                        start=True,
                        stop=True,
                    )
                stg = xsp.tile([128, SLAB, F_OUT], dt.float32)
                nc.scalar.activation(
                    stg[:].rearrange("p s f -> p (s f)"),
                    ps[:].rearrange("p s f -> p (s f)"),
                    mybir.ActivationFunctionType.Copy,
                )
                if t0 < plan.tilesA:
                    tab, tbase, ntil = xA[m], t0, plan.tilesA
                else:
                    tab, tbase, ntil = xB[m], t0 - plan.tilesA, plan.tilesB
                nc.scalar.dma_start(
                    tab[: 128 * ntil, :].rearrange(
                        "(p t) f -> p t f", p=128
                    )[:, tbase : tbase + SLAB, :],
                    stg[:],
                )

            # ---- phase 2+3: gather + segment reduce + scatter, finalize per m ----
            qrr = 0
            order = []
            for m in range(NMP):
                order.append((0, m))
                order.append((1, m))
            for oi, (ch, m) in enumerate(order):
                if True:
                    groups = cfg.g0[m] if ch == 0 else cfg.g1[m]
                    Ts = cfg.T0s[m] if ch == 0 else cfg.T1s[m]
                    tabs = xA if ch == 0 else xB
                    if oi + 1 < len(order):
                        load_stream(*order[oi + 1])
                    acc = accp.tile(
                        [128, plan.NBP, F_OUT], dt.float32, name="acc"
                    )
                    nc.vector.memset(acc[:], 0.0)
                    Bcols = np.zeros(plan.NBP, np.int64)
                    Bcols[1:] = np.cumsum(Ts)[:-1]
                    for (b_lo, b_hi, col_lo, ncols) in groups:
                        it = gst_t[(ch, m)][:, col_lo * 8 : (col_lo + ncols) * 8]
                        gt = gp.tile([128, ncols, F_OUT], dt.float32, name="gt")
                        nc.gpsimd.dma_gather(
                            out_ap=gt[:],
                            in_ap=tabs[m][:],
                            idxs_ap=it,
                            num_idxs=ncols * 128,
                            num_idxs_reg=ncols * 128,
                            elem_size=F_OUT,
                            single_packet=False,
                            queue_num=qrr,
                        )
                        qrr = (qrr + 1) % 4
                        for b in range(b_lo, b_hi):
                            cl = int(Bcols[b] - col_lo)
                            w = int(Ts[b])
                            if w == 0:
                                continue
                            view = gt[:, cl : cl + w, :].rearrange("p t f -> p f t")
                            nc.vector.tensor_reduce(
                                acc[:, b, :], view, mybir.AxisListType.X,
                                mybir.AluOpType.add,
                            )
                    nc.gpsimd.dma_scatter_add(
                        out_ap=agg[m][:],
                        in_ap=acc[:],
                        idxs_ap=si_t[(ch, m)][:],
                        num_idxs=plan.npc_pad,
                        num_idxs_reg=plan.npc,
                        elem_size=F_OUT,
                        single_packet=False,
                        queue_num=qrr,
                    )
                    qrr = (qrr + 1) % 4

                # finalize metapath m after its second chunk:
                # e_m = relu(agg*ri/3 + b/3); mean += e_m
                if ch != 1:
                    continue
                fin = finp.tile([128, plan.NBP, F_OUT], dt.float32, name="fin")
                nc.sync.dma_start(
                    fin[:],
                    agg[m][:].rearrange("(t p) f -> p t f", p=128),
                )
                for b in range(plan.NBP):
                    nc.vector.scalar_tensor_tensor(
                        fin[:, b, :],
                        fin[:, b, :],
                        ri_t[m][:, b : b + 1],
                        ball3_t[:, m * F_OUT : (m + 1) * F_OUT],
                        mybir.AluOpType.mult,
                        mybir.AluOpType.add,
                    )
                    nc.scalar.activation(
                        fin[:, b, :], fin[:, b, :],
                        mybir.ActivationFunctionType.Relu,
                    )
                nc.vector.tensor_tensor(
                    mean_t[:], mean_t[:], fin[:], mybir.AluOpType.add
                )
            nc.sync.dma_start(
                out_d[:].rearrange("(t p) f -> p t f", p=128), mean_t[:]
            )
    nc.compile()
    return nc


_CACHE = {}


def _get_compiled(plan, h, srcs, dsts, Ws, bs):
    cfg, in_maps = _prepare(plan, h, srcs, dsts, Ws, bs)
    key = (
        plan.N,
        plan.ncores,
        tuple(tuple(t) for t in cfg.T0s),
        tuple(tuple(t) for t in cfg.T1s),
    )
    if key not in _CACHE:
        _CACHE[key] = _build_kernel(cfg)
    return _CACHE[key], cfg, in_maps


def run(h, srcs, dsts, Ws, bs, N=None, ncores=8, trace=False):
    N = h.shape[0] if N is None else N
    plan = _make_plan(N, ncores)
    nc, cfg, in_maps = _get_compiled(plan, h, srcs, dsts, Ws, bs)
    res = run_bass_kernel_spmd(
        nc, in_maps, core_ids=list(range(ncores)), trace=trace
    )
    out = np.concatenate(
        [res.results[c]["out"][: plan.npc] for c in range(ncores)], axis=0
    )
    return out[:N], res


def kernel(h, src0, dst0, src1, dst1, src2, dst2, W0, b0, W1, b1, W2, b2):
    h = np.asarray(h, np.float32)
    srcs = [np.asarray(s, np.int64) for s in (src0, src1, src2)]
    dsts = [np.asarray(d, np.int64) for d in (dst0, dst1, dst2)]
    Ws = [np.asarray(w, np.float32) for w in (W0, W1, W2)]
    bs = [np.asarray(b, np.float32) for b in (b0, b1, b2)]
    out, _ = run(h, srcs, dsts, Ws, bs)
    return out.astype(np.float32)



# revision 2
# speedup vs baseline: 1.0334x; 1.0334x over previous
"""HAN layer (3-metapath GCN mean) Trainium2 Bass kernel, 8-core SPMD.

Strategy (dst-sharded pull, h-gather variant):
  - GCN linearity: agg(h*rs_out) @ W == agg((h*rs_out) @ W).  So gather the
    host-prepared bf16 pre-scaled h tables (h * rsqrt(deg_out_m), 128 feats =
    256B rows, the dma_gather minimum elem) and apply W AFTER aggregation.
    This removes the on-device x-table phase entirely: gathers start at t~0
    and the SWDGE descriptor-generation wall (4 queues, ~2.25ns/desc) is the
    only remaining critical path.
  - nodes range-sharded across 8 cores (6250 each); per (core, metapath,
    chunk): in-edges of owned nodes laid out by the host into a gather slot
    schedule: nodes sorted by chunk-degree descending, blocks of 128 nodes,
    per-block fixed column count T[b] (elementwise max over the 8 cores so
    the program is identical across cores).  4096-idx dma_gather calls rotate
    over the 4 SWDGE queues; VectorE tensor_reduce sums each block's columns
    (bf16 in, fp32 out); raw block sums are dma_scatter_add-ed (un-permuting)
    into a zeroed per-metapath DRAM aggregate [npc_pad, 128].
  - final per-metapath pass: per 128-node block, PE-transpose agg, bf16
    matmul by W_m, then rsqrt(deg_in)/3 scale + bias/3 + relu (DVE+ACT),
    accumulated into the mean; one output DMA.  Host concatenates cores.
"""

import numpy as np
import ml_dtypes

import concourse.bass as bass
import concourse.tile as tile
from concourse import bacc, mybir
from concourse.bass_utils import run_bass_kernel_spmd
from concourse.masks import make_identity

F_IN, F_OUT, NMP = 128, 64, 3
GROUP_MAX_COLS = 32  # max slot columns per dma_gather call (4096 idxs)


def _wrap16(flat):
    """slot i -> (partition i%16, free i//16), replicated to 128 partitions."""
    a = flat.astype(np.int16).reshape(-1, 16).T.copy()  # [16, S/16]
    return np.tile(a, (8, 1))


class _NS:
    pass


def _make_plan(N, ncores):
    p = _NS()
    p.N, p.ncores = N, ncores
    p.npc = N // ncores
    p.NBP = (p.npc + 127) // 128
    p.npc_pad = p.NBP * 128
    p.NT = (N + 511) // 512 * 4
    p.N_pad = p.NT * 128
    p.tilesA = (p.NT + 1) // 2
    p.tilesB = p.NT - p.tilesA
    p.CHN = p.tilesA * 128
    p.rowsA = p.tilesA * 128 + 128
    p.rowsB = p.tilesB * 128 + 128
    p.zeroA = p.tilesA * 128
    p.zeroB = p.tilesB * 128
    assert p.rowsA < 32768 and p.rowsB < 32768
    return p


def _build_stream(plan, d_sel, li_sel, order, Ts, zero_base):
    TOT = int(Ts.sum())
    fill = (zero_base + (np.arange(TOT * 128) % 128)).astype(np.int16)
    if TOT == 0 or len(d_sel) == 0:
        return fill
    B = np.zeros(plan.NBP, np.int64)
    B[1:] = np.cumsum(Ts)[:-1]
    rank = np.empty(plan.npc, np.int64)
    rank[order] = np.arange(plan.npc)
    r_e = rank[d_sel]
    o = np.argsort(r_e, kind="stable")
    r_s = r_e[o]
    li = li_sel[o]
    starts = np.searchsorted(r_s, np.arange(plan.npc))
    k = np.arange(len(r_s)) - starts[r_s]
    blk = r_s // 128
    col = B[blk] + k
    assert (k < Ts[blk]).all()
    fill[col * 128 + (r_s % 128)] = li.astype(np.int16)
    return fill


def _groups(Ts):
    out, b, col, NB = [], 0, 0, len(Ts)
    while b < NB:
        if Ts[b] == 0:
            b += 1
            continue
        b_lo, col_lo, ncols = b, col, 0
        while b < NB and Ts[b] > 0 and (ncols == 0 or ncols + Ts[b] <= GROUP_MAX_COLS):
            ncols += int(Ts[b])
            col += int(Ts[b])
            b += 1
        out.append((b_lo, b, col_lo, ncols))
    return out


def _prepare(plan, h, srcs, dsts, Ws, bs):
    N, npc = plan.N, plan.npc

    rs_out, rs_in = [], []
    for m in range(NMP):
        do = np.clip(np.bincount(srcs[m], minlength=N), 1, None).astype(np.float64)
        di = np.clip(np.bincount(dsts[m], minlength=N), 1, None).astype(np.float64)
        rs_out.append((1.0 / np.sqrt(do)).astype(np.float32))
        rs_in.append((1.0 / np.sqrt(di)).astype(np.float32))

    # pre-scaled h chunk tables (bf16), trailing 128 zero rows = padding slots
    hA, hB = [], []
    for m in range(NMP):
        hs = h * rs_out[m][:, None]
        a = np.zeros((plan.rowsA, F_IN), np.float32)
        a[: plan.CHN] = hs[: plan.CHN]
        b_ = np.zeros((plan.rowsB, F_IN), np.float32)
        nb = N - plan.CHN
        b_[:nb] = hs[plan.CHN :]
        hA.append(a.astype(ml_dtypes.bfloat16))
        hB.append(b_.astype(ml_dtypes.bfloat16))

    wall = np.concatenate(Ws, axis=1).astype(ml_dtypes.bfloat16)
    ball3 = np.concatenate(
        [np.tile(b[None, :] / 3.0, (128, 1)) for b in bs], axis=1
    ).astype(np.float32)

    info = {}
    T0s = [np.zeros(plan.NBP, np.int64) for _ in range(NMP)]
    T1s = [np.zeros(plan.NBP, np.int64) for _ in range(NMP)]
    pad = plan.npc_pad - npc
    z = np.zeros(pad, np.int64)
    for c in range(plan.ncores):
        lo = c * npc
        for m in range(NMP):
            sel = (dsts[m] >= lo) & (dsts[m] < lo + npc)
            s = srcs[m][sel]
            d = dsts[m][sel] - lo
            in0 = s < plan.CHN
            c0 = np.bincount(d[in0], minlength=npc)
            c1 = np.bincount(d[~in0], minlength=npc)
            o0 = np.argsort(-c0, kind="stable")
            o1 = np.argsort(-c1, kind="stable")
            info[(c, m)] = (s, d, in0, o0, o1)
            T0s[m] = np.maximum(
                T0s[m], np.concatenate([c0[o0], z]).reshape(plan.NBP, 128).max(1)
            )
            T1s[m] = np.maximum(
                T1s[m], np.concatenate([c1[o1], z]).reshape(plan.NBP, 128).max(1)
            )

    cfg = _NS()
    cfg.plan = plan
    cfg.T0s, cfg.T1s = T0s, T1s
    cfg.g0 = [_groups(T0s[m]) for m in range(NMP)]
    cfg.g1 = [_groups(T1s[m]) for m in range(NMP)]
    cfg.TOT0 = [int(T0s[m].sum()) for m in range(NMP)]
    cfg.TOT1 = [int(T1s[m].sum()) for m in range(NMP)]

    in_maps = []
    for c in range(plan.ncores):
        im = {"wall": wall, "ball3": ball3}
        for m in range(NMP):
            im[f"hA{m}"] = hA[m]
            im[f"hB{m}"] = hB[m]
        lo = c * npc
        for m in range(NMP):
            s, d, in0, o0, o1 = info[(c, m)]
            liA = s[in0]
            liB = s[~in0] - plan.CHN
            st0 = _build_stream(plan, d[in0], liA, o0, T0s[m], plan.zeroA)
            st1 = _build_stream(plan, d[~in0], liB, o1, T1s[m], plan.zeroB)
            im[f"g0_{m}"] = (
                _wrap16(st0) if cfg.TOT0[m] else np.zeros((128, 8), np.int16)
            )
            im[f"g1_{m}"] = (
                _wrap16(st1) if cfg.TOT1[m] else np.zeros((128, 8), np.int16)
            )
            for ch, o in ((0, o0), (1, o1)):
                si = np.full(plan.npc_pad, -1, np.int64)
                si[:npc] = o
                im[f"si{ch}_{m}"] = _wrap16(si)
            # natural-order rsqrt(deg_in)/3 per (partition, block)
            full = np.zeros(plan.npc_pad, np.float32)
            full[:npc] = rs_in[m][lo : lo + npc] / 3.0
            im[f"ri_{m}"] = full.reshape(plan.NBP, 128).T.copy()
        in_maps.append(im)
    return cfg, in_maps


def _build_kernel(cfg):
    plan = cfg.plan
    nc = bacc.Bacc(
        "TRN2",
        target_bir_lowering=False,
        debug=False,
        num_devices=plan.ncores,
        num_swdge_queues=4,
    )
    dt = mybir.dt
    hA_d, hB_d = {}, {}
    for m in range(NMP):
        hA_d[m] = nc.dram_tensor(f"hA{m}", (plan.rowsA, F_IN), dt.bfloat16, kind="ExternalInput").ap()
        hB_d[m] = nc.dram_tensor(f"hB{m}", (plan.rowsB, F_IN), dt.bfloat16, kind="ExternalInput").ap()
    wall_d = nc.dram_tensor("wall", (128, F_OUT * NMP), dt.bfloat16, kind="ExternalInput").ap()
    ball3_d = nc.dram_tensor("ball3", (128, F_OUT * NMP), dt.float32, kind="ExternalInput").ap()
    g_d, si_d, ri_d, agg = {}, {}, {}, {}
    for m in range(NMP):
        w0 = max(cfg.TOT0[m] * 8, 8)
        w1 = max(cfg.TOT1[m] * 8, 8)
        g_d[(0, m)] = nc.dram_tensor(f"g0_{m}", (128, w0), dt.int16, kind="ExternalInput").ap()
        g_d[(1, m)] = nc.dram_tensor(f"g1_{m}", (128, w1), dt.int16, kind="ExternalInput").ap()
        si_d[(0, m)] = nc.dram_tensor(f"si0_{m}", (128, plan.npc_pad // 16), dt.int16, kind="ExternalInput").ap()
        si_d[(1, m)] = nc.dram_tensor(f"si1_{m}", (128, plan.npc_pad // 16), dt.int16, kind="ExternalInput").ap()
        ri_d[m] = nc.dram_tensor(f"ri_{m}", (128, plan.NBP), dt.float32, kind="ExternalInput").ap()
        agg[m] = nc.dram_tensor(f"agg_{m}", (plan.npc_pad, F_IN), dt.float32, kind="Internal").ap()
    out_d = nc.dram_tensor("out", (plan.npc_pad, F_OUT), dt.float32, kind="ExternalOutput").ap()

    with tile.TileContext(nc) as tc:
        with tc.tile_pool(name="const", bufs=1) as constp, \
             tc.tile_pool(name="ps", bufs=2, space="PSUM") as psp, \
             tc.tile_pool(name="psw", bufs=2, space="PSUM") as pswp, \
             tc.tile_pool(name="gst", bufs=2) as gstp, \
             tc.tile_pool(name="gath", bufs=6) as gp, \
             tc.tile_pool(name="accs", bufs=2) as accp, \
             tc.tile_pool(name="fin", bufs=2) as finp, \
             tc.tile_pool(name="agg_ld", bufs=2) as aglp, \
             tc.tile_pool(name="aggT", bufs=2) as agtp, \
             tc.tile_pool(name="mean", bufs=1) as meanp:

            # idx mega-stream preload for the first (m=0, chunk A) stream so
            # gathers start immediately.
            gst_t = {}

            def load_stream(ch, m):
                tot = cfg.TOT0[m] if ch == 0 else cfg.TOT1[m]
                w = max(tot * 8, 8)
                t = gstp.tile([128, w], dt.int16, name="gst")
                nc.sync.dma_start(t[:], g_d[(ch, m)][:, :w])
                gst_t[(ch, m)] = t

            load_stream(0, 0)

            wall_t = constp.tile([128, F_OUT * NMP], dt.bfloat16)
            nc.sync.dma_start(wall_t[:], wall_d[:])
            ball3_t = constp.tile([128, F_OUT * NMP], dt.float32)
            nc.sync.dma_start(ball3_t[:], ball3_d[:])
            ident = constp.tile([128, 128], dt.float32)
            make_identity(nc, ident[:])
            ri_t, si_t = {}, {}
            for m in range(NMP):
                ri_t[m] = constp.tile([128, plan.NBP], dt.float32, name=f"ri_t{m}")
                nc.sync.dma_start(ri_t[m][:], ri_d[m][:])
                for ch in (0, 1):
                    si_t[(ch, m)] = constp.tile(
                        [128, plan.npc_pad // 16], dt.int16, name=f"si_t{ch}_{m}"
                    )
                    nc.sync.dma_start(si_t[(ch, m)][:], si_d[(ch, m)][:])

            # zero the DRAM aggregates (scatter_add accumulates into them);
            # borrow an acc-pool buffer so no extra SBUF is held.
            zt = accp.tile([128, plan.NBP, F_IN], dt.float32, name="acc")
            nc.vector.memset(zt[:], 0.0)
            for m in range(NMP):
                nc.sync.dma_start(
                    agg[m][:].rearrange("n f -> (n f)").rearrange(
                        "(p x) -> p x", p=128
                    ),
                    zt[:].rearrange("p t f -> p (t f)"),
                )
            mean_t = meanp.tile([128, plan.NBP, F_OUT], dt.float32)
            nc.vector.memset(mean_t[:], 0.0)

            qrr = 0

            def do_scatter(ch, m, acc):
                nonlocal qrr
                nc.gpsimd.dma_scatter_add(
                    out_ap=agg[m][:],
                    in_ap=acc[:],
                    idxs_ap=si_t[(ch, m)][:],
                    num_idxs=plan.npc_pad,
                    num_idxs_reg=plan.npc,
                    elem_size=F_IN,
                    single_packet=False,
                    queue_num=qrr,
                )
                qrr = (qrr + 1) % 4

            # finalize metapath m (called deferred, after both scatters):
            # e_m = relu((agg @ W_m)*ri/3 + b/3); mean += e_m
            def finalize(m):
                for b in range(plan.NBP):
                    ag = aglp.tile([128, F_IN], dt.float32, name="ag")
                    nc.sync.dma_start(
                        ag[:], agg[m][b * 128 : (b + 1) * 128, :]
                    )
                    psT = psp.tile([128, 128], dt.float32, space="PSUM")
                    nc.tensor.transpose(out=psT[:], in_=ag[:], identity=ident[:])
                    agT = agtp.tile([128, 128], dt.bfloat16, name="agT")
                    nc.scalar.activation(
                        agT[:], psT[:], mybir.ActivationFunctionType.Copy
                    )
                    ps2 = pswp.tile([128, F_OUT], dt.float32, space="PSUM")
                    nc.tensor.matmul(
                        ps2[:],
                        lhsT=agT[:],
                        rhs=wall_t[:, m * F_OUT : (m + 1) * F_OUT],
                        start=True,
                        stop=True,
                    )
                    fin = finp.tile([128, F_OUT], dt.float32, name="fin")
                    nc.vector.scalar_tensor_tensor(
                        fin[:],
                        ps2[:],
                        ri_t[m][:, b : b + 1],
                        ball3_t[:, m * F_OUT : (m + 1) * F_OUT],
                        mybir.AluOpType.mult,
                        mybir.AluOpType.add,
                    )
                    nc.scalar.activation(
                        fin[:], fin[:], mybir.ActivationFunctionType.Relu
                    )
                    nc.vector.tensor_tensor(
                        mean_t[:, b, :], mean_t[:, b, :], fin[:],
                        mybir.AluOpType.add,
                    )
            nc.sync.dma_start(
                out_d[:].rearrange("(t p) f -> p t f", p=128), mean_t[:]
            )

            order = []
            for m in range(NMP):
                order.append((0, m))
                order.append((1, m))
            pending = []
            for oi, (ch, m) in enumerate(order):
                groups = cfg.g0[m] if ch == 0 else cfg.g1[m]
                Ts = cfg.T0s[m] if ch == 0 else cfg.T1s[m]
                tab = hA_d[m] if ch == 0 else hB_d[m]
                if oi + 1 < len(order):
                    load_stream(*order[oi + 1])
                acc = accp.tile([128, plan.NBP, F_IN], dt.float32, name="acc")
                nc.vector.memset(acc[:], 0.0)
                Bcols = np.zeros(plan.NBP, np.int64)
                Bcols[1:] = np.cumsum(Ts)[:-1]
                for gi, (b_lo, b_hi, col_lo, ncols) in enumerate(groups):
                    if gi == 6 and pending:
                        # previous chunk's scatter/finalize issue here so its
                        # vector-sem wait sits behind 6 queued gathers instead
                        # of stalling the gpsimd engine head-of-line.
                        for f_ in pending:
                            f_()
                        pending = []
                    it = gst_t[(ch, m)][:, col_lo * 8 : (col_lo + ncols) * 8]
                    gt = gp.tile([128, ncols, F_IN], dt.bfloat16, name="gt")
                    nc.gpsimd.dma_gather(
                        out_ap=gt[:],
                        in_ap=tab[:],
                        idxs_ap=it,
                        num_idxs=ncols * 128,
                        num_idxs_reg=ncols * 128,
                        elem_size=F_IN,
                        single_packet=False,
                        queue_num=qrr,
                    )
                    qrr = (qrr + 1) % 4
                    for b in range(b_lo, b_hi):
                        cl = int(Bcols[b] - col_lo)
                        w = int(Ts[b])
                        if w == 0:
                            continue
                        # pairwise-fold wide blocks with contiguous bf16 adds
                        # (2 elem/cyc) before the strided reduce (~3.7 cyc/elem)
                        while w >= 8:
                            if w % 2:
                                nc.vector.tensor_tensor(
                                    gt[:, cl, :], gt[:, cl, :],
                                    gt[:, cl + w - 1, :], mybir.AluOpType.add,
                                )
                                w -= 1
                            h = w // 2
                            nc.vector.tensor_tensor(
                                gt[:, cl : cl + h, :],
                                gt[:, cl : cl + h, :],
                                gt[:, cl + h : cl + 2 * h, :],
                                mybir.AluOpType.add,
                            )
                            w = h
                        view = gt[:, cl : cl + w, :].rearrange("p t f -> p f t")
                        nc.vector.tensor_reduce(
                            acc[:, b, :], view, mybir.AxisListType.X,
                            mybir.AluOpType.add,
                        )
                for f_ in pending:
                    f_()
                pending = []
                pending.append(
                    (lambda ch=ch, m=m, acc=acc: do_scatter(ch, m, acc))
                )
                if ch == 1:
                    pending.append(lambda m=m: finalize(m))
            for f_ in pending:
                f_()

            nc.sync.dma_start(
                out_d[:].rearrange("(t p) f -> p t f", p=128), mean_t[:]
            )

    nc.compile()
    return nc


_CACHE = {}


def _get_compiled(plan, h, srcs, dsts, Ws, bs):
    cfg, in_maps = _prepare(plan, h, srcs, dsts, Ws, bs)
    key = (
        plan.N,
        plan.ncores,
        tuple(tuple(t) for t in cfg.T0s),
        tuple(tuple(t) for t in cfg.T1s),
    )
    if key not in _CACHE:
        _CACHE[key] = _build_kernel(cfg)
    return _CACHE[key], cfg, in_maps


def run(h, srcs, dsts, Ws, bs, N=None, ncores=8, trace=False):
    N = h.shape[0] if N is None else N
    plan = _make_plan(N, ncores)
    nc, cfg, in_maps = _get_compiled(plan, h, srcs, dsts, Ws, bs)
    res = run_bass_kernel_spmd(
        nc, in_maps, core_ids=list(range(ncores)), trace=trace
    )
    out = np.concatenate(
        [res.results[c]["out"][: plan.npc] for c in range(ncores)], axis=0
    )
    return out[:N], res


def kernel(h, src0, dst0, src1, dst1, src2, dst2, W0, b0, W1, b1, W2, b2):
    h = np.asarray(h, np.float32)
    srcs = [np.asarray(s, np.int64) for s in (src0, src1, src2)]
    dsts = [np.asarray(d, np.int64) for d in (dst0, dst1, dst2)]
    Ws = [np.asarray(w, np.float32) for w in (W0, W1, W2)]
    bs = [np.asarray(b, np.float32) for b in (b0, b1, b2)]
    out, _ = run(h, srcs, dsts, Ws, bs)
    return out.astype(np.float32)


# revision 3
# speedup vs baseline: 1.0458x; 1.0120x over previous
"""HAN layer (3-metapath GCN mean) Trainium2 Bass kernel, 8-core SPMD.

Strategy (dst-sharded pull, h-gather variant):
  - GCN linearity: agg(h*rs_out) @ W == agg((h*rs_out) @ W).  So gather the
    host-prepared bf16 pre-scaled h tables (h * rsqrt(deg_out_m), 128 feats =
    256B rows, the dma_gather minimum elem) and apply W AFTER aggregation.
    This removes the on-device x-table phase entirely: gathers start at t~0
    and the SWDGE descriptor-generation wall (4 queues, ~2.25ns/desc) is the
    only remaining critical path.
  - nodes range-sharded across 8 cores (6250 each); per (core, metapath,
    chunk): in-edges of owned nodes laid out by the host into a gather slot
    schedule: nodes sorted by chunk-degree descending, blocks of 128 nodes,
    per-block fixed column count T[b] (elementwise max over the 8 cores so
    the program is identical across cores).  4096-idx dma_gather calls rotate
    over the 4 SWDGE queues; VectorE tensor_reduce sums each block's columns
    (bf16 in, fp32 out); raw block sums are dma_scatter_add-ed (un-permuting)
    into a zeroed per-metapath DRAM aggregate [npc_pad, 128].
  - final per-metapath pass: per 128-node block, PE-transpose agg, bf16
    matmul by W_m, then rsqrt(deg_in)/3 scale + bias/3 + relu (DVE+ACT),
    accumulated into the mean; one output DMA.  Host concatenates cores.
"""

import numpy as np
import ml_dtypes

import concourse.bass as bass
import concourse.tile as tile
from concourse import bacc, mybir
from concourse.bass_utils import run_bass_kernel_spmd
from concourse.masks import make_identity

F_IN, F_OUT, NMP = 128, 64, 3
GROUP_MAX_COLS = 32  # max slot columns per dma_gather call (4096 idxs)


def _wrap16(flat):
    """slot i -> (partition i%16, free i//16), replicated to 128 partitions."""
    a = flat.astype(np.int16).reshape(-1, 16).T.copy()  # [16, S/16]
    return np.tile(a, (8, 1))


class _NS:
    pass


def _make_plan(N, ncores):
    p = _NS()
    p.N, p.ncores = N, ncores
    p.npc = N // ncores
    p.NBP = (p.npc + 127) // 128
    p.npc_pad = p.NBP * 128
    p.NT = (N + 511) // 512 * 4
    p.N_pad = p.NT * 128
    p.tilesA = (p.NT + 1) // 2
    p.tilesB = p.NT - p.tilesA
    p.CHN = p.tilesA * 128
    p.rowsA = p.tilesA * 128 + 128
    p.rowsB = p.tilesB * 128 + 128
    p.zeroA = p.tilesA * 128
    p.zeroB = p.tilesB * 128
    assert p.rowsA < 32768 and p.rowsB < 32768
    return p


def _build_stream(plan, d_sel, li_sel, order, Ts, zero_base):
    TOT = int(Ts.sum())
    fill = (zero_base + (np.arange(TOT * 128) % 128)).astype(np.int16)
    if TOT == 0 or len(d_sel) == 0:
        return fill
    B = np.zeros(plan.NBP, np.int64)
    B[1:] = np.cumsum(Ts)[:-1]
    rank = np.empty(plan.npc, np.int64)
    rank[order] = np.arange(plan.npc)
    r_e = rank[d_sel]
    o = np.argsort(r_e, kind="stable")
    r_s = r_e[o]
    li = li_sel[o]
    starts = np.searchsorted(r_s, np.arange(plan.npc))
    k = np.arange(len(r_s)) - starts[r_s]
    blk = r_s // 128
    col = B[blk] + k
    assert (k < Ts[blk]).all()
    fill[col * 128 + (r_s % 128)] = li.astype(np.int16)
    return fill


def _groups(Ts):
    out, b, col, NB = [], 0, 0, len(Ts)
    while b < NB:
        if Ts[b] == 0:
            b += 1
            continue
        b_lo, col_lo, ncols = b, col, 0
        while b < NB and Ts[b] > 0 and (ncols == 0 or ncols + Ts[b] <= GROUP_MAX_COLS):
            ncols += int(Ts[b])
            col += int(Ts[b])
            b += 1
        out.append((b_lo, b, col_lo, ncols))
    return out


def _prepare(plan, h, srcs, dsts, Ws, bs):
    N, npc = plan.N, plan.npc

    rs_out, rs_in = [], []
    for m in range(NMP):
        do = np.clip(np.bincount(srcs[m], minlength=N), 1, None).astype(np.float64)
        di = np.clip(np.bincount(dsts[m], minlength=N), 1, None).astype(np.float64)
        rs_out.append((1.0 / np.sqrt(do)).astype(np.float32))
        rs_in.append((1.0 / np.sqrt(di)).astype(np.float32))

    # pre-scaled h chunk tables (bf16), trailing 128 zero rows = padding slots
    hA, hB = [], []
    for m in range(NMP):
        hs = h * rs_out[m][:, None]
        a = np.zeros((plan.rowsA, F_IN), np.float32)
        a[: plan.CHN] = hs[: plan.CHN]
        b_ = np.zeros((plan.rowsB, F_IN), np.float32)
        nb = N - plan.CHN
        b_[:nb] = hs[plan.CHN :]
        hA.append(a.astype(ml_dtypes.bfloat16))
        hB.append(b_.astype(ml_dtypes.bfloat16))

    wall = np.concatenate(Ws, axis=1).astype(ml_dtypes.bfloat16)
    ball3 = np.concatenate(
        [np.tile(b[None, :] / 3.0, (128, 1)) for b in bs], axis=1
    ).astype(np.float32)

    info = {}
    T0s = [np.zeros(plan.NBP, np.int64) for _ in range(NMP)]
    T1s = [np.zeros(plan.NBP, np.int64) for _ in range(NMP)]
    pad = plan.npc_pad - npc
    z = np.zeros(pad, np.int64)
    for c in range(plan.ncores):
        lo = c * npc
        for m in range(NMP):
            sel = (dsts[m] >= lo) & (dsts[m] < lo + npc)
            s = srcs[m][sel]
            d = dsts[m][sel] - lo
            in0 = s < plan.CHN
            c0 = np.bincount(d[in0], minlength=npc)
            c1 = np.bincount(d[~in0], minlength=npc)
            o0 = np.argsort(-c0, kind="stable")
            o1 = np.argsort(-c1, kind="stable")
            info[(c, m)] = (s, d, in0, o0, o1)
            T0s[m] = np.maximum(
                T0s[m], np.concatenate([c0[o0], z]).reshape(plan.NBP, 128).max(1)
            )
            T1s[m] = np.maximum(
                T1s[m], np.concatenate([c1[o1], z]).reshape(plan.NBP, 128).max(1)
            )

    cfg = _NS()
    cfg.plan = plan
    cfg.T0s, cfg.T1s = T0s, T1s
    cfg.g0 = [_groups(T0s[m]) for m in range(NMP)]
    cfg.g1 = [_groups(T1s[m]) for m in range(NMP)]
    cfg.TOT0 = [int(T0s[m].sum()) for m in range(NMP)]
    cfg.TOT1 = [int(T1s[m].sum()) for m in range(NMP)]

    in_maps = []
    for c in range(plan.ncores):
        im = {"wall": wall, "ball3": ball3}
        for m in range(NMP):
            im[f"hA{m}"] = hA[m]
            im[f"hB{m}"] = hB[m]
        lo = c * npc
        for m in range(NMP):
            s, d, in0, o0, o1 = info[(c, m)]
            liA = s[in0]
            liB = s[~in0] - plan.CHN
            st0 = _build_stream(plan, d[in0], liA, o0, T0s[m], plan.zeroA)
            st1 = _build_stream(plan, d[~in0], liB, o1, T1s[m], plan.zeroB)
            im[f"g0_{m}"] = (
                _wrap16(st0) if cfg.TOT0[m] else np.zeros((128, 8), np.int16)
            )
            im[f"g1_{m}"] = (
                _wrap16(st1) if cfg.TOT1[m] else np.zeros((128, 8), np.int16)
            )
            for ch, o in ((0, o0), (1, o1)):
                si = np.full(plan.npc_pad, -1, np.int64)
                si[:npc] = o
                im[f"si{ch}_{m}"] = _wrap16(si)
            # natural-order rsqrt(deg_in)/3 per (partition, block)
            full = np.zeros(plan.npc_pad, np.float32)
            full[:npc] = rs_in[m][lo : lo + npc] / 3.0
            im[f"ri_{m}"] = full.reshape(plan.NBP, 128).T.copy()
        in_maps.append(im)
    return cfg, in_maps


def _build_kernel(cfg):
    plan = cfg.plan
    nc = bacc.Bacc(
        "TRN2",
        target_bir_lowering=False,
        debug=False,
        num_devices=plan.ncores,
        num_swdge_queues=4,
    )
    dt = mybir.dt
    hA_d, hB_d = {}, {}
    for m in range(NMP):
        hA_d[m] = nc.dram_tensor(f"hA{m}", (plan.rowsA, F_IN), dt.bfloat16, kind="ExternalInput").ap()
        hB_d[m] = nc.dram_tensor(f"hB{m}", (plan.rowsB, F_IN), dt.bfloat16, kind="ExternalInput").ap()
    wall_d = nc.dram_tensor("wall", (128, F_OUT * NMP), dt.bfloat16, kind="ExternalInput").ap()
    ball3_d = nc.dram_tensor("ball3", (128, F_OUT * NMP), dt.float32, kind="ExternalInput").ap()
    g_d, si_d, ri_d, agg = {}, {}, {}, {}
    for m in range(NMP):
        w0 = max(cfg.TOT0[m] * 8, 8)
        w1 = max(cfg.TOT1[m] * 8, 8)
        g_d[(0, m)] = nc.dram_tensor(f"g0_{m}", (128, w0), dt.int16, kind="ExternalInput").ap()
        g_d[(1, m)] = nc.dram_tensor(f"g1_{m}", (128, w1), dt.int16, kind="ExternalInput").ap()
        si_d[(0, m)] = nc.dram_tensor(f"si0_{m}", (128, plan.npc_pad // 16), dt.int16, kind="ExternalInput").ap()
        si_d[(1, m)] = nc.dram_tensor(f"si1_{m}", (128, plan.npc_pad // 16), dt.int16, kind="ExternalInput").ap()
        ri_d[m] = nc.dram_tensor(f"ri_{m}", (128, plan.NBP), dt.float32, kind="ExternalInput").ap()
        agg[m] = nc.dram_tensor(f"agg_{m}", (plan.npc_pad, F_IN), dt.float32, kind="Internal").ap()
    out_d = nc.dram_tensor("out", (plan.npc_pad, F_OUT), dt.float32, kind="ExternalOutput").ap()

    with tile.TileContext(nc) as tc:
        with tc.tile_pool(name="const", bufs=1) as constp, \
             tc.tile_pool(name="ps", bufs=2, space="PSUM") as psp, \
             tc.tile_pool(name="psw", bufs=2, space="PSUM") as pswp, \
             tc.tile_pool(name="gst", bufs=2) as gstp, \
             tc.tile_pool(name="gath", bufs=6) as gp, \
             tc.tile_pool(name="accs", bufs=2) as accp, \
             tc.tile_pool(name="fin", bufs=2) as finp, \
             tc.tile_pool(name="agg_ld", bufs=2) as aglp, \
             tc.tile_pool(name="aggT", bufs=2) as agtp, \
             tc.tile_pool(name="mean", bufs=1) as meanp:

            # idx mega-stream preload for the first (m=0, chunk A) stream so
            # gathers start immediately.
            gst_t = {}

            def load_stream(ch, m):
                tot = cfg.TOT0[m] if ch == 0 else cfg.TOT1[m]
                w = max(tot * 8, 8)
                t = gstp.tile([128, w], dt.int16, name="gst")
                nc.sync.dma_start(t[:], g_d[(ch, m)][:, :w])
                gst_t[(ch, m)] = t

            load_stream(0, 0)

            wall_t = constp.tile([128, F_OUT * NMP], dt.bfloat16)
            nc.sync.dma_start(wall_t[:], wall_d[:])
            ball3_t = constp.tile([128, F_OUT * NMP], dt.float32)
            nc.sync.dma_start(ball3_t[:], ball3_d[:])
            ident = constp.tile([128, 128], dt.float32)
            make_identity(nc, ident[:])
            ri_t, si_t = {}, {}
            for m in range(NMP):
                ri_t[m] = constp.tile([128, plan.NBP], dt.float32, name=f"ri_t{m}")
                nc.sync.dma_start(ri_t[m][:], ri_d[m][:])
                for ch in (0, 1):
                    si_t[(ch, m)] = constp.tile(
                        [128, plan.npc_pad // 16], dt.int16, name=f"si_t{ch}_{m}"
                    )
                    nc.sync.dma_start(si_t[(ch, m)][:], si_d[(ch, m)][:])

            # zero the DRAM aggregates (scatter_add accumulates into them);
            # borrow an acc-pool buffer so no extra SBUF is held.
            zt = accp.tile([128, plan.NBP, F_IN], dt.float32, name="acc")
            nc.vector.memset(zt[:], 0.0)
            for m in range(NMP):
                nc.sync.dma_start(
                    agg[m][:].rearrange("n f -> (n f)").rearrange(
                        "(p x) -> p x", p=128
                    ),
                    zt[:].rearrange("p t f -> p (t f)"),
                )
            mean_t = meanp.tile([128, plan.NBP, F_OUT], dt.float32)
            nc.vector.memset(mean_t[:], 0.0)

            qrr = 0

            def do_scatter(ch, m, acc):
                # split across the 4 SWDGE queues (block-aligned pieces) so
                # the per-queue DGE walls shrink 4x, cutting the tail stall.
                nonlocal qrr
                NBP = plan.NBP
                bounds = [0, (NBP + 3) // 4, (NBP + 1) // 2, (3 * NBP) // 4, NBP]
                for pi in range(4):
                    b0, b1 = bounds[pi], bounds[pi + 1]
                    if b1 <= b0:
                        continue
                    s0, s1 = b0 * 128, b1 * 128
                    valid = max(0, min(plan.npc, s1) - s0)
                    if valid == 0:
                        continue
                    nc.gpsimd.dma_scatter_add(
                        out_ap=agg[m][:],
                        in_ap=acc[:, b0:b1, :],
                        idxs_ap=si_t[(ch, m)][:, b0 * 8 : b1 * 8],
                        num_idxs=s1 - s0,
                        num_idxs_reg=valid,
                        elem_size=F_IN,
                        single_packet=False,
                        queue_num=qrr,
                    )
                    qrr = (qrr + 1) % 4

            # finalize metapath m (called deferred, after both scatters):
            # e_m = relu((agg @ W_m)*ri/3 + b/3); mean += e_m
            def finalize(m):
                for b in range(plan.NBP):
                    ag = aglp.tile([128, F_IN], dt.float32, name="ag")
                    nc.sync.dma_start(
                        ag[:], agg[m][b * 128 : (b + 1) * 128, :]
                    )
                    psT = psp.tile([128, 128], dt.float32, space="PSUM")
                    nc.tensor.transpose(out=psT[:], in_=ag[:], identity=ident[:])
                    agT = agtp.tile([128, 128], dt.bfloat16, name="agT")
                    nc.scalar.activation(
                        agT[:], psT[:], mybir.ActivationFunctionType.Copy
                    )
                    ps2 = pswp.tile([128, F_OUT], dt.float32, space="PSUM")
                    nc.tensor.matmul(
                        ps2[:],
                        lhsT=agT[:],
                        rhs=wall_t[:, m * F_OUT : (m + 1) * F_OUT],
                        start=True,
                        stop=True,
                    )
                    fin = finp.tile([128, F_OUT], dt.float32, name="fin")
                    nc.vector.scalar_tensor_tensor(
                        fin[:],
                        ps2[:],
                        ri_t[m][:, b : b + 1],
                        ball3_t[:, m * F_OUT : (m + 1) * F_OUT],
                        mybir.AluOpType.mult,
                        mybir.AluOpType.add,
                    )
                    nc.scalar.activation(
                        fin[:], fin[:], mybir.ActivationFunctionType.Relu
                    )
                    nc.vector.tensor_tensor(
                        mean_t[:, b, :], mean_t[:, b, :], fin[:],
                        mybir.AluOpType.add,
                    )
            nc.sync.dma_start(
                out_d[:].rearrange("(t p) f -> p t f", p=128), mean_t[:]
            )

            order = []
            for m in range(NMP):
                order.append((0, m))
                order.append((1, m))
            pending = []
            for oi, (ch, m) in enumerate(order):
                groups = cfg.g0[m] if ch == 0 else cfg.g1[m]
                Ts = cfg.T0s[m] if ch == 0 else cfg.T1s[m]
                tab = hA_d[m] if ch == 0 else hB_d[m]
                if oi + 1 < len(order):
                    load_stream(*order[oi + 1])
                acc = accp.tile([128, plan.NBP, F_IN], dt.float32, name="acc")
                nc.vector.memset(acc[:], 0.0)
                Bcols = np.zeros(plan.NBP, np.int64)
                Bcols[1:] = np.cumsum(Ts)[:-1]
                for gi, (b_lo, b_hi, col_lo, ncols) in enumerate(groups):
                    if gi == 6 and pending:
                        # previous chunk's scatter/finalize issue here so its
                        # vector-sem wait sits behind 6 queued gathers instead
                        # of stalling the gpsimd engine head-of-line.
                        for f_ in pending:
                            f_()
                        pending = []
                    it = gst_t[(ch, m)][:, col_lo * 8 : (col_lo + ncols) * 8]
                    gt = gp.tile([128, ncols, F_IN], dt.bfloat16, name="gt")
                    nc.gpsimd.dma_gather(
                        out_ap=gt[:],
                        in_ap=tab[:],
                        idxs_ap=it,
                        num_idxs=ncols * 128,
                        num_idxs_reg=ncols * 128,
                        elem_size=F_IN,
                        single_packet=False,
                        queue_num=qrr,
                    )
                    qrr = (qrr + 1) % 4
                    for b in range(b_lo, b_hi):
                        cl = int(Bcols[b] - col_lo)
                        w = int(Ts[b])
                        if w == 0:
                            continue
                        # pairwise-fold wide blocks with contiguous bf16 adds
                        # (2 elem/cyc) before the strided reduce (~3.7 cyc/elem)
                        while w >= 4:
                            if w % 2:
                                nc.vector.tensor_tensor(
                                    gt[:, cl, :], gt[:, cl, :],
                                    gt[:, cl + w - 1, :], mybir.AluOpType.add,
                                )
                                w -= 1
                            h = w // 2
                            nc.vector.tensor_tensor(
                                gt[:, cl : cl + h, :],
                                gt[:, cl : cl + h, :],
                                gt[:, cl + h : cl + 2 * h, :],
                                mybir.AluOpType.add,
                            )
                            w = h
                        view = gt[:, cl : cl + w, :].rearrange("p t f -> p f t")
                        nc.vector.tensor_reduce(
                            acc[:, b, :], view, mybir.AxisListType.X,
                            mybir.AluOpType.add,
                        )
                for f_ in pending:
                    f_()
                pending = []
                pending.append(
                    (lambda ch=ch, m=m, acc=acc: do_scatter(ch, m, acc))
                )
                if ch == 1:
                    pending.append(lambda m=m: finalize(m))
            for f_ in pending:
                f_()

            nc.sync.dma_start(
                out_d[:].rearrange("(t p) f -> p t f", p=128), mean_t[:]
            )

    nc.compile()
    return nc


_CACHE = {}


def _get_compiled(plan, h, srcs, dsts, Ws, bs):
    cfg, in_maps = _prepare(plan, h, srcs, dsts, Ws, bs)
    key = (
        plan.N,
        plan.ncores,
        tuple(tuple(t) for t in cfg.T0s),
        tuple(tuple(t) for t in cfg.T1s),
    )
    if key not in _CACHE:
        _CACHE[key] = _build_kernel(cfg)
    return _CACHE[key], cfg, in_maps


def run(h, srcs, dsts, Ws, bs, N=None, ncores=8, trace=False):
    N = h.shape[0] if N is None else N
    plan = _make_plan(N, ncores)
    nc, cfg, in_maps = _get_compiled(plan, h, srcs, dsts, Ws, bs)
    res = run_bass_kernel_spmd(
        nc, in_maps, core_ids=list(range(ncores)), trace=trace
    )
    out = np.concatenate(
        [res.results[c]["out"][: plan.npc] for c in range(ncores)], axis=0
    )
    return out[:N], res


def kernel(h, src0, dst0, src1, dst1, src2, dst2, W0, b0, W1, b1, W2, b2):
    h = np.asarray(h, np.float32)
    srcs = [np.asarray(s, np.int64) for s in (src0, src1, src2)]
    dsts = [np.asarray(d, np.int64) for d in (dst0, dst1, dst2)]
    Ws = [np.asarray(w, np.float32) for w in (W0, W1, W2)]
    bs = [np.asarray(b, np.float32) for b in (b0, b1, b2)]
    out, _ = run(h, srcs, dsts, Ws, bs)
    return out.astype(np.float32)


# revision 4
# speedup vs baseline: 1.0839x; 1.0365x over previous
"""HAN layer (3-metapath GCN mean) Trainium2 Bass kernel, 8-core SPMD.

Strategy (dst-sharded pull, h-gather variant):
  - GCN linearity: agg(h*rs_out) @ W == agg((h*rs_out) @ W).  So gather the
    host-prepared bf16 pre-scaled h tables (h * rsqrt(deg_out_m), 128 feats =
    256B rows, the dma_gather minimum elem) and apply W AFTER aggregation.
    This removes the on-device x-table phase entirely: gathers start at t~0
    and the SWDGE descriptor-generation wall (4 queues, ~2.25ns/desc) is the
    only remaining critical path.
  - nodes range-sharded across 8 cores (6250 each); per (core, metapath,
    chunk): in-edges of owned nodes laid out by the host into a gather slot
    schedule: nodes sorted by chunk-degree descending, blocks of 128 nodes,
    per-block fixed column count T[b] (elementwise max over the 8 cores so
    the program is identical across cores).  4096-idx dma_gather calls rotate
    over the 4 SWDGE queues; VectorE tensor_reduce sums each block's columns
    (bf16 in, fp32 out); raw block sums are dma_scatter_add-ed (un-permuting)
    into a zeroed per-metapath DRAM aggregate [npc_pad, 128].
  - final per-metapath pass: per 128-node block, PE-transpose agg, bf16
    matmul by W_m, then rsqrt(deg_in)/3 scale + bias/3 + relu (DVE+ACT),
    accumulated into the mean; one output DMA.  Host concatenates cores.
"""

import numpy as np
import ml_dtypes

import concourse.bass as bass
import concourse.tile as tile
from concourse import bacc, mybir
from concourse.bass_utils import run_bass_kernel_spmd
from concourse.masks import make_identity

F_IN, F_OUT, NMP = 128, 64, 3
GROUP_MAX_COLS = 32  # max slot columns per dma_gather call (4096 idxs)


def _wrap16(flat):
    """slot i -> (partition i%16, free i//16), replicated to 128 partitions."""
    a = flat.astype(np.int16).reshape(-1, 16).T.copy()  # [16, S/16]
    return np.tile(a, (8, 1))


class _NS:
    pass


def _make_plan(N, ncores):
    p = _NS()
    p.N, p.ncores = N, ncores
    p.npc = N // ncores
    p.NBP = (p.npc + 127) // 128
    p.npc_pad = p.NBP * 128
    p.NT = (N + 511) // 512 * 4
    p.N_pad = p.NT * 128
    p.tilesA = (p.NT + 1) // 2
    p.tilesB = p.NT - p.tilesA
    p.CHN = p.tilesA * 128
    p.rowsA = p.tilesA * 128 + 128
    p.rowsB = p.tilesB * 128 + 128
    p.zeroA = p.tilesA * 128
    p.zeroB = p.tilesB * 128
    assert p.rowsA < 32768 and p.rowsB < 32768
    return p


def _build_stream(plan, d_sel, li_sel, order, Ts, zero_base):
    TOT = int(Ts.sum())
    fill = (zero_base + (np.arange(TOT * 128) % 128)).astype(np.int16)
    if TOT == 0 or len(d_sel) == 0:
        return fill
    B = np.zeros(plan.NBP, np.int64)
    B[1:] = np.cumsum(Ts)[:-1]
    rank = np.empty(plan.npc, np.int64)
    rank[order] = np.arange(plan.npc)
    r_e = rank[d_sel]
    o = np.argsort(r_e, kind="stable")
    r_s = r_e[o]
    li = li_sel[o]
    starts = np.searchsorted(r_s, np.arange(plan.npc))
    k = np.arange(len(r_s)) - starts[r_s]
    blk = r_s // 128
    col = B[blk] + k
    assert (k < Ts[blk]).all()
    fill[col * 128 + (r_s % 128)] = li.astype(np.int16)
    return fill


def _groups(Ts):
    out, b, col, NB = [], 0, 0, len(Ts)
    while b < NB:
        if Ts[b] == 0:
            b += 1
            continue
        b_lo, col_lo, ncols = b, col, 0
        while b < NB and Ts[b] > 0 and (ncols == 0 or ncols + Ts[b] <= GROUP_MAX_COLS):
            ncols += int(Ts[b])
            col += int(Ts[b])
            b += 1
        out.append((b_lo, b, col_lo, ncols))
    return out


def _prepare(plan, h, srcs, dsts, Ws, bs):
    N, npc = plan.N, plan.npc

    rs_out, rs_in = [], []
    for m in range(NMP):
        do = np.clip(np.bincount(srcs[m], minlength=N), 1, None).astype(np.float64)
        di = np.clip(np.bincount(dsts[m], minlength=N), 1, None).astype(np.float64)
        rs_out.append((1.0 / np.sqrt(do)).astype(np.float32))
        rs_in.append((1.0 / np.sqrt(di)).astype(np.float32))

    # pre-scaled h chunk tables (bf16), trailing 128 zero rows = padding slots
    hA, hB = [], []
    for m in range(NMP):
        hs = h * rs_out[m][:, None]
        a = np.zeros((plan.rowsA, F_IN), np.float32)
        a[: plan.CHN] = hs[: plan.CHN]
        b_ = np.zeros((plan.rowsB, F_IN), np.float32)
        nb = N - plan.CHN
        b_[:nb] = hs[plan.CHN :]
        hA.append(a.astype(ml_dtypes.bfloat16))
        hB.append(b_.astype(ml_dtypes.bfloat16))

    wall = np.concatenate(Ws, axis=1).astype(ml_dtypes.bfloat16)
    ball3 = np.concatenate(
        [np.tile(b[None, :] / 3.0, (128, 1)) for b in bs], axis=1
    ).astype(np.float32)

    info = {}
    T0s = [np.zeros(plan.NBP, np.int64) for _ in range(NMP)]
    T1s = [np.zeros(plan.NBP, np.int64) for _ in range(NMP)]
    pad = plan.npc_pad - npc
    z = np.zeros(pad, np.int64)
    for c in range(plan.ncores):
        lo = c * npc
        for m in range(NMP):
            sel = (dsts[m] >= lo) & (dsts[m] < lo + npc)
            s = srcs[m][sel]
            d = dsts[m][sel] - lo
            in0 = s < plan.CHN
            c0 = np.bincount(d[in0], minlength=npc)
            c1 = np.bincount(d[~in0], minlength=npc)
            o0 = np.argsort(-c0, kind="stable")
            o1 = np.argsort(-c1, kind="stable")
            info[(c, m)] = (s, d, in0, o0, o1)
            T0s[m] = np.maximum(
                T0s[m], np.concatenate([c0[o0], z]).reshape(plan.NBP, 128).max(1)
            )
            T1s[m] = np.maximum(
                T1s[m], np.concatenate([c1[o1], z]).reshape(plan.NBP, 128).max(1)
            )

    cfg = _NS()
    cfg.plan = plan
    cfg.T0s, cfg.T1s = T0s, T1s
    cfg.g0 = [_groups(T0s[m]) for m in range(NMP)]
    cfg.g1 = [_groups(T1s[m]) for m in range(NMP)]
    cfg.TOT0 = [int(T0s[m].sum()) for m in range(NMP)]
    cfg.TOT1 = [int(T1s[m].sum()) for m in range(NMP)]

    in_maps = []
    for c in range(plan.ncores):
        im = {"wall": wall, "ball3": ball3}
        for m in range(NMP):
            im[f"hA{m}"] = hA[m]
            im[f"hB{m}"] = hB[m]
        lo = c * npc
        for m in range(NMP):
            s, d, in0, o0, o1 = info[(c, m)]
            liA = s[in0]
            liB = s[~in0] - plan.CHN
            st0 = _build_stream(plan, d[in0], liA, o0, T0s[m], plan.zeroA)
            st1 = _build_stream(plan, d[~in0], liB, o1, T1s[m], plan.zeroB)
            im[f"g0_{m}"] = (
                _wrap16(st0) if cfg.TOT0[m] else np.zeros((128, 8), np.int16)
            )
            im[f"g1_{m}"] = (
                _wrap16(st1) if cfg.TOT1[m] else np.zeros((128, 8), np.int16)
            )
            for ch, o in ((0, o0), (1, o1)):
                si = np.full(plan.npc_pad, -1, np.int64)
                si[:npc] = o
                im[f"si{ch}_{m}"] = _wrap16(si)
            # natural-order rsqrt(deg_in)/3 per (partition, block)
            full = np.zeros(plan.npc_pad, np.float32)
            full[:npc] = rs_in[m][lo : lo + npc] / 3.0
            im[f"ri_{m}"] = full.reshape(plan.NBP, 128).T.copy()
        in_maps.append(im)
    return cfg, in_maps


def _build_kernel(cfg):
    plan = cfg.plan
    nc = bacc.Bacc(
        "TRN2",
        target_bir_lowering=False,
        debug=False,
        num_devices=plan.ncores,
        num_swdge_queues=4,
    )
    dt = mybir.dt
    hA_d, hB_d = {}, {}
    for m in range(NMP):
        hA_d[m] = nc.dram_tensor(f"hA{m}", (plan.rowsA, F_IN), dt.bfloat16, kind="ExternalInput").ap()
        hB_d[m] = nc.dram_tensor(f"hB{m}", (plan.rowsB, F_IN), dt.bfloat16, kind="ExternalInput").ap()
    wall_d = nc.dram_tensor("wall", (128, F_OUT * NMP), dt.bfloat16, kind="ExternalInput").ap()
    ball3_d = nc.dram_tensor("ball3", (128, F_OUT * NMP), dt.float32, kind="ExternalInput").ap()
    g_d, si_d, ri_d, agg = {}, {}, {}, {}
    for m in range(NMP):
        w0 = max(cfg.TOT0[m] * 8, 8)
        w1 = max(cfg.TOT1[m] * 8, 8)
        g_d[(0, m)] = nc.dram_tensor(f"g0_{m}", (128, w0), dt.int16, kind="ExternalInput").ap()
        g_d[(1, m)] = nc.dram_tensor(f"g1_{m}", (128, w1), dt.int16, kind="ExternalInput").ap()
        si_d[(0, m)] = nc.dram_tensor(f"si0_{m}", (128, plan.npc_pad // 16), dt.int16, kind="ExternalInput").ap()
        si_d[(1, m)] = nc.dram_tensor(f"si1_{m}", (128, plan.npc_pad // 16), dt.int16, kind="ExternalInput").ap()
        ri_d[m] = nc.dram_tensor(f"ri_{m}", (128, plan.NBP), dt.float32, kind="ExternalInput").ap()
        agg[m] = nc.dram_tensor(f"agg_{m}", (plan.npc_pad, F_IN), dt.float32, kind="Internal").ap()
    out_d = nc.dram_tensor("out", (plan.npc_pad, F_OUT), dt.float32, kind="ExternalOutput").ap()

    with tile.TileContext(nc) as tc:
        with tc.tile_pool(name="const", bufs=1) as constp, \
             tc.tile_pool(name="ps", bufs=4, space="PSUM") as psp, \
             tc.tile_pool(name="psw", bufs=4, space="PSUM") as pswp, \
             tc.tile_pool(name="gst", bufs=2) as gstp, \
             tc.tile_pool(name="gath", bufs=6) as gp, \
             tc.tile_pool(name="accs", bufs=2) as accp, \
             tc.tile_pool(name="fin", bufs=4) as finp, \
             tc.tile_pool(name="agg_ld", bufs=3) as aglp, \
             tc.tile_pool(name="aggT", bufs=4) as agtp, \
             tc.tile_pool(name="mean", bufs=1) as meanp:

            # idx mega-stream preload for the first (m=0, chunk A) stream so
            # gathers start immediately.
            gst_t = {}

            def load_stream(ch, m):
                tot = cfg.TOT0[m] if ch == 0 else cfg.TOT1[m]
                w = max(tot * 8, 8)
                t = gstp.tile([128, w], dt.int16, name="gst")
                nc.sync.dma_start(t[:], g_d[(ch, m)][:, :w])
                gst_t[(ch, m)] = t

            load_stream(0, 0)

            wall_t = constp.tile([128, F_OUT * NMP], dt.bfloat16)
            nc.sync.dma_start(wall_t[:], wall_d[:])
            ball3_t = constp.tile([128, F_OUT * NMP], dt.float32)
            nc.sync.dma_start(ball3_t[:], ball3_d[:])
            ident = constp.tile([128, 128], dt.float32)
            make_identity(nc, ident[:])
            ri_t, si_t = {}, {}
            for m in range(NMP):
                ri_t[m] = constp.tile([128, plan.NBP], dt.float32, name=f"ri_t{m}")
                nc.sync.dma_start(ri_t[m][:], ri_d[m][:])
                for ch in (0, 1):
                    si_t[(ch, m)] = constp.tile(
                        [128, plan.npc_pad // 16], dt.int16, name=f"si_t{ch}_{m}"
                    )
                    nc.sync.dma_start(si_t[(ch, m)][:], si_d[(ch, m)][:])

            # zero the DRAM aggregates (scatter_add accumulates into them);
            # borrow an acc-pool buffer so no extra SBUF is held.
            zt = accp.tile([128, plan.NBP, F_IN], dt.float32, name="acc")
            nc.vector.memset(zt[:], 0.0)
            for m in range(NMP):
                nc.sync.dma_start(
                    agg[m][:].rearrange("n f -> (n f)").rearrange(
                        "(p x) -> p x", p=128
                    ),
                    zt[:].rearrange("p t f -> p (t f)"),
                )
            mean_t = meanp.tile([128, plan.NBP, F_OUT], dt.float32)
            nc.vector.memset(mean_t[:], 0.0)

            qrr = 0

            def do_scatter(ch, m, acc):
                # split across the 4 SWDGE queues (block-aligned pieces) so
                # the per-queue DGE walls shrink 4x, cutting the tail stall.
                nonlocal qrr
                NBP = plan.NBP
                bounds = [0, (NBP + 3) // 4, (NBP + 1) // 2, (3 * NBP) // 4, NBP]
                for pi in range(4):
                    b0, b1 = bounds[pi], bounds[pi + 1]
                    if b1 <= b0:
                        continue
                    s0, s1 = b0 * 128, b1 * 128
                    valid = max(0, min(plan.npc, s1) - s0)
                    if valid == 0:
                        continue
                    nc.gpsimd.dma_scatter_add(
                        out_ap=agg[m][:],
                        in_ap=acc[:, b0:b1, :],
                        idxs_ap=si_t[(ch, m)][:, b0 * 8 : b1 * 8],
                        num_idxs=s1 - s0,
                        num_idxs_reg=valid,
                        elem_size=F_IN,
                        single_packet=False,
                        queue_num=qrr,
                    )
                    qrr = (qrr + 1) % 4

            # finalize metapath m (called deferred, after both scatters):
            # e_m = relu((agg @ W_m)*ri/3 + b/3); mean += e_m
            def finalize(m):
                for b0 in range(0, plan.NBP, 4):
                    nb4 = min(4, plan.NBP - b0)
                    ag = aglp.tile([128, 4, F_IN], dt.float32, name="ag")
                    nc.sync.dma_start(
                        ag[:, :nb4, :],
                        agg[m][b0 * 128 : (b0 + nb4) * 128, :].rearrange(
                            "(t p) f -> p t f", p=128
                        ),
                    )
                    for b in range(b0, b0 + nb4):
                        _fin_block(m, b, ag[:, b - b0, :])

            def _fin_block(m, b, ag):
                    psT = psp.tile([128, 128], dt.float32, space="PSUM")
                    nc.tensor.transpose(out=psT[:], in_=ag[:], identity=ident[:])
                    agT = agtp.tile([128, 128], dt.bfloat16, name="agT")
                    nc.scalar.activation(
                        agT[:], psT[:], mybir.ActivationFunctionType.Copy
                    )
                    ps2 = pswp.tile([128, F_OUT], dt.float32, space="PSUM")
                    nc.tensor.matmul(
                        ps2[:],
                        lhsT=agT[:],
                        rhs=wall_t[:, m * F_OUT : (m + 1) * F_OUT],
                        start=True,
                        stop=True,
                    )
                    fin = finp.tile([128, F_OUT], dt.float32, name="fin")
                    nc.vector.scalar_tensor_tensor(
                        fin[:],
                        ps2[:],
                        ri_t[m][:, b : b + 1],
                        ball3_t[:, m * F_OUT : (m + 1) * F_OUT],
                        mybir.AluOpType.mult,
                        mybir.AluOpType.add,
                    )
                    nc.scalar.activation(
                        fin[:], fin[:], mybir.ActivationFunctionType.Relu
                    )
                    nc.vector.tensor_tensor(
                        mean_t[:, b, :], mean_t[:, b, :], fin[:],
                        mybir.AluOpType.add,
                    )
            nc.sync.dma_start(
                out_d[:].rearrange("(t p) f -> p t f", p=128), mean_t[:]
            )

            order = []
            for m in range(NMP):
                order.append((0, m))
                order.append((1, m))
            pending = []
            for oi, (ch, m) in enumerate(order):
                groups = cfg.g0[m] if ch == 0 else cfg.g1[m]
                Ts = cfg.T0s[m] if ch == 0 else cfg.T1s[m]
                tab = hA_d[m] if ch == 0 else hB_d[m]
                if oi + 1 < len(order):
                    load_stream(*order[oi + 1])
                acc = accp.tile([128, plan.NBP, F_IN], dt.float32, name="acc")
                nc.vector.memset(acc[:], 0.0)
                Bcols = np.zeros(plan.NBP, np.int64)
                Bcols[1:] = np.cumsum(Ts)[:-1]
                for gi, (b_lo, b_hi, col_lo, ncols) in enumerate(groups):
                    if gi == 6 and pending:
                        # previous chunk's scatter/finalize issue here so its
                        # vector-sem wait sits behind 6 queued gathers instead
                        # of stalling the gpsimd engine head-of-line.
                        for f_ in pending:
                            f_()
                        pending = []
                    it = gst_t[(ch, m)][:, col_lo * 8 : (col_lo + ncols) * 8]
                    gt = gp.tile([128, ncols, F_IN], dt.bfloat16, name="gt")
                    nc.gpsimd.dma_gather(
                        out_ap=gt[:],
                        in_ap=tab[:],
                        idxs_ap=it,
                        num_idxs=ncols * 128,
                        num_idxs_reg=ncols * 128,
                        elem_size=F_IN,
                        single_packet=False,
                        queue_num=qrr,
                    )
                    qrr = (qrr + 1) % 4
                    for b in range(b_lo, b_hi):
                        cl = int(Bcols[b] - col_lo)
                        w = int(Ts[b])
                        if w == 0:
                            continue
                        # pairwise-fold wide blocks with contiguous bf16 adds
                        # (2 elem/cyc) before the strided reduce (~3.7 cyc/elem)
                        while w >= 4:
                            if w % 2:
                                nc.vector.tensor_tensor(
                                    gt[:, cl, :], gt[:, cl, :],
                                    gt[:, cl + w - 1, :], mybir.AluOpType.add,
                                )
                                w -= 1
                            h = w // 2
                            nc.vector.tensor_tensor(
                                gt[:, cl : cl + h, :],
                                gt[:, cl : cl + h, :],
                                gt[:, cl + h : cl + 2 * h, :],
                                mybir.AluOpType.add,
                            )
                            w = h
                        view = gt[:, cl : cl + w, :].rearrange("p t f -> p f t")
                        nc.vector.tensor_reduce(
                            acc[:, b, :], view, mybir.AxisListType.X,
                            mybir.AluOpType.add,
                        )
                for f_ in pending:
                    f_()
                pending = []
                pending.append(
                    (lambda ch=ch, m=m, acc=acc: do_scatter(ch, m, acc))
                )
                if ch == 1:
                    pending.append(lambda m=m: finalize(m))
            for f_ in pending:
                f_()

            nc.sync.dma_start(
                out_d[:].rearrange("(t p) f -> p t f", p=128), mean_t[:]
            )

    nc.compile()
    return nc


_CACHE = {}


def _get_compiled(plan, h, srcs, dsts, Ws, bs):
    cfg, in_maps = _prepare(plan, h, srcs, dsts, Ws, bs)
    key = (
        plan.N,
        plan.ncores,
        tuple(tuple(t) for t in cfg.T0s),
        tuple(tuple(t) for t in cfg.T1s),
    )
    if key not in _CACHE:
        _CACHE[key] = _build_kernel(cfg)
    return _CACHE[key], cfg, in_maps


def run(h, srcs, dsts, Ws, bs, N=None, ncores=8, trace=False):
    N = h.shape[0] if N is None else N
    plan = _make_plan(N, ncores)
    nc, cfg, in_maps = _get_compiled(plan, h, srcs, dsts, Ws, bs)
    res = run_bass_kernel_spmd(
        nc, in_maps, core_ids=list(range(ncores)), trace=trace
    )
    out = np.concatenate(
        [res.results[c]["out"][: plan.npc] for c in range(ncores)], axis=0
    )
    return out[:N], res


def kernel(h, src0, dst0, src1, dst1, src2, dst2, W0, b0, W1, b1, W2, b2):
    h = np.asarray(h, np.float32)
    srcs = [np.asarray(s, np.int64) for s in (src0, src1, src2)]
    dsts = [np.asarray(d, np.int64) for d in (dst0, dst1, dst2)]
    Ws = [np.asarray(w, np.float32) for w in (W0, W1, W2)]
    bs = [np.asarray(b, np.float32) for b in (b0, b1, b2)]
    out, _ = run(h, srcs, dsts, Ws, bs)
    return out.astype(np.float32)


# revision 5
# speedup vs baseline: 1.1201x; 1.0334x over previous
"""HAN layer (3-metapath GCN mean) Trainium2 Bass kernel, 8-core SPMD.

Strategy (dst-sharded pull, h-gather variant):
  - GCN linearity: agg(h*rs_out) @ W == agg((h*rs_out) @ W).  So gather the
    host-prepared bf16 pre-scaled h tables (h * rsqrt(deg_out_m), 128 feats =
    256B rows, the dma_gather minimum elem) and apply W AFTER aggregation.
    This removes the on-device x-table phase entirely: gathers start at t~0
    and the SWDGE descriptor-generation wall (4 queues, ~2.25ns/desc) is the
    only remaining critical path.
  - nodes range-sharded across 8 cores (6250 each); per (core, metapath,
    chunk): in-edges of owned nodes laid out by the host into a gather slot
    schedule: nodes sorted by chunk-degree descending, blocks of 128 nodes,
    per-block fixed column count T[b] (elementwise max over the 8 cores so
    the program is identical across cores).  4096-idx dma_gather calls rotate
    over the 4 SWDGE queues; VectorE tensor_reduce sums each block's columns
    (bf16 in, fp32 out); raw block sums are dma_scatter_add-ed (un-permuting)
    into a zeroed per-metapath DRAM aggregate [npc_pad, 128].
  - final per-metapath pass: per 128-node block, PE-transpose agg, bf16
    matmul by W_m, then rsqrt(deg_in)/3 scale + bias/3 + relu (DVE+ACT),
    accumulated into the mean; one output DMA.  Host concatenates cores.
"""

import numpy as np
import ml_dtypes

import concourse.bass as bass
import concourse.tile as tile
from concourse import bacc, mybir
from concourse.bass_utils import run_bass_kernel_spmd
from concourse.masks import make_identity

F_IN, F_OUT, NMP = 128, 64, 3
GROUP_MAX_COLS = 32  # max slot columns per dma_gather call (4096 idxs)


def _wrap16(flat):
    """slot i -> (partition i%16, free i//16), replicated to 128 partitions."""
    a = flat.astype(np.int16).reshape(-1, 16).T.copy()  # [16, S/16]
    return np.tile(a, (8, 1))


class _NS:
    pass


def _make_plan(N, ncores):
    p = _NS()
    p.N, p.ncores = N, ncores
    p.npc = N // ncores
    p.NBP = (p.npc + 127) // 128
    p.npc_pad = p.NBP * 128
    p.NT = (N + 511) // 512 * 4
    p.N_pad = p.NT * 128
    p.tilesA = (p.NT + 1) // 2
    p.tilesB = p.NT - p.tilesA
    p.CHN = p.tilesA * 128
    p.rowsA = p.tilesA * 128 + 128
    p.rowsB = p.tilesB * 128 + 128
    p.zeroA = p.tilesA * 128
    p.zeroB = p.tilesB * 128
    assert p.rowsA < 32768 and p.rowsB < 32768
    return p


def _build_stream(plan, d_sel, li_sel, order, Ts, zero_base):
    TOT = int(Ts.sum())
    fill = (zero_base + (np.arange(TOT * 128) % 128)).astype(np.int16)
    if TOT == 0 or len(d_sel) == 0:
        return fill
    B = np.zeros(plan.NBP, np.int64)
    B[1:] = np.cumsum(Ts)[:-1]
    rank = np.empty(plan.npc, np.int64)
    rank[order] = np.arange(plan.npc)
    r_e = rank[d_sel]
    o = np.argsort(r_e, kind="stable")
    r_s = r_e[o]
    li = li_sel[o]
    starts = np.searchsorted(r_s, np.arange(plan.npc))
    k = np.arange(len(r_s)) - starts[r_s]
    blk = r_s // 128
    col = B[blk] + k
    assert (k < Ts[blk]).all()
    fill[col * 128 + (r_s % 128)] = li.astype(np.int16)
    return fill


def _groups(Ts):
    out, b, col, NB = [], 0, 0, len(Ts)
    while b < NB:
        if Ts[b] == 0:
            b += 1
            continue
        b_lo, col_lo, ncols = b, col, 0
        while b < NB and Ts[b] > 0 and (ncols == 0 or ncols + Ts[b] <= GROUP_MAX_COLS):
            ncols += int(Ts[b])
            col += int(Ts[b])
            b += 1
        out.append((b_lo, b, col_lo, ncols))
    return out


def _prepare(plan, h, srcs, dsts, Ws, bs):
    N, npc = plan.N, plan.npc

    rs_out, rs_in = [], []
    for m in range(NMP):
        do = np.clip(np.bincount(srcs[m], minlength=N), 1, None).astype(np.float64)
        di = np.clip(np.bincount(dsts[m], minlength=N), 1, None).astype(np.float64)
        rs_out.append((1.0 / np.sqrt(do)).astype(np.float32))
        rs_in.append((1.0 / np.sqrt(di)).astype(np.float32))

    # pre-scaled h chunk tables (bf16), trailing 128 zero rows = padding slots
    hA, hB = [], []
    for m in range(NMP):
        hs = h * rs_out[m][:, None]
        a = np.zeros((plan.rowsA, F_IN), np.float32)
        a[: plan.CHN] = hs[: plan.CHN]
        b_ = np.zeros((plan.rowsB, F_IN), np.float32)
        nb = N - plan.CHN
        b_[:nb] = hs[plan.CHN :]
        hA.append(a.astype(ml_dtypes.bfloat16))
        hB.append(b_.astype(ml_dtypes.bfloat16))

    wall = np.concatenate(Ws, axis=1).astype(ml_dtypes.bfloat16)
    ball3 = np.concatenate(
        [np.tile(b[None, :] / 3.0, (128, 1)) for b in bs], axis=1
    ).astype(np.float32)

    info = {}
    T0s = [np.zeros(plan.NBP, np.int64) for _ in range(NMP)]
    T1s = [np.zeros(plan.NBP, np.int64) for _ in range(NMP)]
    pad = plan.npc_pad - npc
    z = np.zeros(pad, np.int64)
    for c in range(plan.ncores):
        lo = c * npc
        for m in range(NMP):
            sel = (dsts[m] >= lo) & (dsts[m] < lo + npc)
            s = srcs[m][sel]
            d = dsts[m][sel] - lo
            in0 = s < plan.CHN
            c0 = np.bincount(d[in0], minlength=npc)
            c1 = np.bincount(d[~in0], minlength=npc)
            o0 = np.argsort(-c0, kind="stable")
            o1 = np.argsort(-c1, kind="stable")
            info[(c, m)] = (s, d, in0, o0, o1)
            T0s[m] = np.maximum(
                T0s[m], np.concatenate([c0[o0], z]).reshape(plan.NBP, 128).max(1)
            )
            T1s[m] = np.maximum(
                T1s[m], np.concatenate([c1[o1], z]).reshape(plan.NBP, 128).max(1)
            )

    cfg = _NS()
    cfg.plan = plan
    cfg.T0s, cfg.T1s = T0s, T1s
    cfg.g0 = [_groups(T0s[m]) for m in range(NMP)]
    cfg.g1 = [_groups(T1s[m]) for m in range(NMP)]
    cfg.TOT0 = [int(T0s[m].sum()) for m in range(NMP)]
    cfg.TOT1 = [int(T1s[m].sum()) for m in range(NMP)]

    in_maps = []
    for c in range(plan.ncores):
        im = {"wall": wall, "ball3": ball3}
        for m in range(NMP):
            im[f"hA{m}"] = hA[m]
            im[f"hB{m}"] = hB[m]
        lo = c * npc
        for m in range(NMP):
            s, d, in0, o0, o1 = info[(c, m)]
            liA = s[in0]
            liB = s[~in0] - plan.CHN
            st0 = _build_stream(plan, d[in0], liA, o0, T0s[m], plan.zeroA)
            st1 = _build_stream(plan, d[~in0], liB, o1, T1s[m], plan.zeroB)
            im[f"g0_{m}"] = (
                _wrap16(st0) if cfg.TOT0[m] else np.zeros((128, 8), np.int16)
            )
            im[f"g1_{m}"] = (
                _wrap16(st1) if cfg.TOT1[m] else np.zeros((128, 8), np.int16)
            )
            for ch, o in ((0, o0), (1, o1)):
                si = np.full(plan.npc_pad, -1, np.int64)
                si[:npc] = o
                im[f"si{ch}_{m}"] = _wrap16(si)
            # natural-order rsqrt(deg_in)/3 per (partition, block)
            full = np.zeros(plan.npc_pad, np.float32)
            full[:npc] = rs_in[m][lo : lo + npc] / 3.0
            im[f"ri_{m}"] = full.reshape(plan.NBP, 128).T.copy()
        in_maps.append(im)
    return cfg, in_maps


def _build_kernel(cfg):
    plan = cfg.plan
    nc = bacc.Bacc(
        "TRN2",
        target_bir_lowering=False,
        debug=False,
        num_devices=plan.ncores,
        num_swdge_queues=4,
    )
    dt = mybir.dt
    hA_d, hB_d = {}, {}
    for m in range(NMP):
        hA_d[m] = nc.dram_tensor(f"hA{m}", (plan.rowsA, F_IN), dt.bfloat16, kind="ExternalInput").ap()
        hB_d[m] = nc.dram_tensor(f"hB{m}", (plan.rowsB, F_IN), dt.bfloat16, kind="ExternalInput").ap()
    wall_d = nc.dram_tensor("wall", (128, F_OUT * NMP), dt.bfloat16, kind="ExternalInput").ap()
    ball3_d = nc.dram_tensor("ball3", (128, F_OUT * NMP), dt.float32, kind="ExternalInput").ap()
    g_d, si_d, ri_d, agg = {}, {}, {}, {}
    for m in range(NMP):
        w0 = max(cfg.TOT0[m] * 8, 8)
        w1 = max(cfg.TOT1[m] * 8, 8)
        g_d[(0, m)] = nc.dram_tensor(f"g0_{m}", (128, w0), dt.int16, kind="ExternalInput").ap()
        g_d[(1, m)] = nc.dram_tensor(f"g1_{m}", (128, w1), dt.int16, kind="ExternalInput").ap()
        si_d[(0, m)] = nc.dram_tensor(f"si0_{m}", (128, plan.npc_pad // 16), dt.int16, kind="ExternalInput").ap()
        si_d[(1, m)] = nc.dram_tensor(f"si1_{m}", (128, plan.npc_pad // 16), dt.int16, kind="ExternalInput").ap()
        ri_d[m] = nc.dram_tensor(f"ri_{m}", (128, plan.NBP), dt.float32, kind="ExternalInput").ap()
        agg[m] = nc.dram_tensor(f"agg_{m}", (plan.npc_pad, F_IN), dt.float32, kind="Internal").ap()
    out_d = nc.dram_tensor("out", (plan.npc_pad, F_OUT), dt.float32, kind="ExternalOutput").ap()

    with tile.TileContext(nc) as tc:
        with tc.tile_pool(name="const", bufs=1) as constp, \
             tc.tile_pool(name="ps", bufs=4, space="PSUM") as psp, \
             tc.tile_pool(name="psw", bufs=4, space="PSUM") as pswp, \
             tc.tile_pool(name="gst", bufs=3) as gstp, \
             tc.tile_pool(name="gath", bufs=7) as gp, \
             tc.tile_pool(name="accs", bufs=2) as accp, \
             tc.tile_pool(name="fin", bufs=4) as finp, \
             tc.tile_pool(name="agg_ld", bufs=3) as aglp, \
             tc.tile_pool(name="aggT", bufs=4) as agtp, \
             tc.tile_pool(name="mean", bufs=1) as meanp:

            # idx mega-stream preload for the first (m=0, chunk A) stream so
            # gathers start immediately.
            gst_t = {}

            def load_stream(ch, m):
                tot = cfg.TOT0[m] if ch == 0 else cfg.TOT1[m]
                w = max(tot * 8, 8)
                t = gstp.tile([128, w], dt.int16, name="gst")
                nc.sync.dma_start(t[:], g_d[(ch, m)][:, :w])
                gst_t[(ch, m)] = t

            load_stream(0, 0)

            wall_t = constp.tile([128, F_OUT * NMP], dt.bfloat16)
            nc.sync.dma_start(wall_t[:], wall_d[:])
            ball3_t = constp.tile([128, F_OUT * NMP], dt.float32)
            nc.sync.dma_start(ball3_t[:], ball3_d[:])
            ident = constp.tile([128, 128], dt.float32)
            make_identity(nc, ident[:])
            ri_t, si_t = {}, {}
            for m in range(NMP):
                ri_t[m] = constp.tile([128, plan.NBP], dt.float32, name=f"ri_t{m}")
                nc.sync.dma_start(ri_t[m][:], ri_d[m][:])
                for ch in (0, 1):
                    si_t[(ch, m)] = constp.tile(
                        [128, plan.npc_pad // 16], dt.int16, name=f"si_t{ch}_{m}"
                    )
                    nc.sync.dma_start(si_t[(ch, m)][:], si_d[(ch, m)][:])

            # zero the DRAM aggregates (scatter_add accumulates into them);
            # borrow an acc-pool buffer so no extra SBUF is held.
            zt = accp.tile([128, plan.NBP, F_IN], dt.float32, name="acc")
            nc.vector.memset(zt[:], 0.0)
            for m in range(NMP):
                nc.sync.dma_start(
                    agg[m][:].rearrange("n f -> (n f)").rearrange(
                        "(p x) -> p x", p=128
                    ),
                    zt[:].rearrange("p t f -> p (t f)"),
                )
            mean_t = meanp.tile([128, plan.NBP, F_OUT], dt.float32)
            nc.vector.memset(mean_t[:], 0.0)

            qrr = 0

            def do_scatter(ch, m, acc):
                # split across the 4 SWDGE queues (block-aligned pieces) so
                # the per-queue DGE walls shrink 4x, cutting the tail stall.
                nonlocal qrr
                NBP = plan.NBP
                bounds = [0, (NBP + 3) // 4, (NBP + 1) // 2, (3 * NBP) // 4, NBP]
                for pi in range(4):
                    b0, b1 = bounds[pi], bounds[pi + 1]
                    if b1 <= b0:
                        continue
                    s0, s1 = b0 * 128, b1 * 128
                    valid = max(0, min(plan.npc, s1) - s0)
                    if valid == 0:
                        continue
                    nc.gpsimd.dma_scatter_add(
                        out_ap=agg[m][:],
                        in_ap=acc[:, b0:b1, :],
                        idxs_ap=si_t[(ch, m)][:, b0 * 8 : b1 * 8],
                        num_idxs=s1 - s0,
                        num_idxs_reg=valid,
                        elem_size=F_IN,
                        single_packet=False,
                        queue_num=qrr,
                    )
                    qrr = (qrr + 1) % 4

            # finalize metapath m (called deferred, after both scatters):
            # e_m = relu((agg @ W_m)*ri/3 + b/3); mean += e_m
            def finalize(m):
                for b0 in range(0, plan.NBP, 4):
                    nb4 = min(4, plan.NBP - b0)
                    ag = aglp.tile([128, 4, F_IN], dt.float32, name="ag")
                    nc.sync.dma_start(
                        ag[:, :nb4, :],
                        agg[m][b0 * 128 : (b0 + nb4) * 128, :].rearrange(
                            "(t p) f -> p t f", p=128
                        ),
                    )
                    for b in range(b0, b0 + nb4):
                        _fin_block(m, b, ag[:, b - b0, :])

            def _fin_block(m, b, ag):
                    psT = psp.tile([128, 128], dt.float32, space="PSUM")
                    nc.tensor.transpose(out=psT[:], in_=ag[:], identity=ident[:])
                    agT = agtp.tile([128, 128], dt.bfloat16, name="agT")
                    nc.scalar.activation(
                        agT[:], psT[:], mybir.ActivationFunctionType.Copy
                    )
                    ps2 = pswp.tile([128, F_OUT], dt.float32, space="PSUM")
                    nc.tensor.matmul(
                        ps2[:],
                        lhsT=agT[:],
                        rhs=wall_t[:, m * F_OUT : (m + 1) * F_OUT],
                        start=True,
                        stop=True,
                    )
                    fin = finp.tile([128, F_OUT], dt.float32, name="fin")
                    nc.vector.scalar_tensor_tensor(
                        fin[:],
                        ps2[:],
                        ri_t[m][:, b : b + 1],
                        ball3_t[:, m * F_OUT : (m + 1) * F_OUT],
                        mybir.AluOpType.mult,
                        mybir.AluOpType.add,
                    )
                    nc.scalar.activation(
                        fin[:], fin[:], mybir.ActivationFunctionType.Relu
                    )
                    nc.vector.tensor_tensor(
                        mean_t[:, b, :], mean_t[:, b, :], fin[:],
                        mybir.AluOpType.add,
                    )
            nc.sync.dma_start(
                out_d[:].rearrange("(t p) f -> p t f", p=128), mean_t[:]
            )

            order = []
            for m in range(NMP):
                order.append((0, m))
                order.append((1, m))
            pending = []
            for oi, (ch, m) in enumerate(order):
                groups = cfg.g0[m] if ch == 0 else cfg.g1[m]
                Ts = cfg.T0s[m] if ch == 0 else cfg.T1s[m]
                tab = hA_d[m] if ch == 0 else hB_d[m]
                if oi + 1 < len(order):
                    load_stream(*order[oi + 1])
                acc = accp.tile([128, plan.NBP, F_IN], dt.float32, name="acc")
                nc.vector.memset(acc[:], 0.0)
                Bcols = np.zeros(plan.NBP, np.int64)
                Bcols[1:] = np.cumsum(Ts)[:-1]
                for gi, (b_lo, b_hi, col_lo, ncols) in enumerate(groups):
                    if gi == 6 and pending:
                        # previous chunk's scatter/finalize issue here so its
                        # vector-sem wait sits behind 6 queued gathers instead
                        # of stalling the gpsimd engine head-of-line.
                        for f_ in pending:
                            f_()
                        pending = []
                    it = gst_t[(ch, m)][:, col_lo * 8 : (col_lo + ncols) * 8]
                    gt = gp.tile([128, ncols, F_IN], dt.bfloat16, name="gt")
                    nc.gpsimd.dma_gather(
                        out_ap=gt[:],
                        in_ap=tab[:],
                        idxs_ap=it,
                        num_idxs=ncols * 128,
                        num_idxs_reg=ncols * 128,
                        elem_size=F_IN,
                        single_packet=False,
                        queue_num=qrr,
                    )
                    qrr = (qrr + 1) % 4
                    for b in range(b_lo, b_hi):
                        cl = int(Bcols[b] - col_lo)
                        w = int(Ts[b])
                        if w == 0:
                            continue
                        # pairwise-fold wide blocks with contiguous bf16 adds
                        # (2 elem/cyc) before the strided reduce (~3.7 cyc/elem)
                        while w >= 4:
                            if w % 2:
                                nc.vector.tensor_tensor(
                                    gt[:, cl, :], gt[:, cl, :],
                                    gt[:, cl + w - 1, :], mybir.AluOpType.add,
                                )
                                w -= 1
                            h = w // 2
                            nc.vector.tensor_tensor(
                                gt[:, cl : cl + h, :],
                                gt[:, cl : cl + h, :],
                                gt[:, cl + h : cl + 2 * h, :],
                                mybir.AluOpType.add,
                            )
                            w = h
                        view = gt[:, cl : cl + w, :].rearrange("p t f -> p f t")
                        nc.vector.tensor_reduce(
                            acc[:, b, :], view, mybir.AxisListType.X,
                            mybir.AluOpType.add,
                        )
                for f_ in pending:
                    f_()
                pending = []
                pending.append(
                    (lambda ch=ch, m=m, acc=acc: do_scatter(ch, m, acc))
                )
                if ch == 1:
                    pending.append(lambda m=m: finalize(m))
            for f_ in pending:
                f_()

            nc.sync.dma_start(
                out_d[:].rearrange("(t p) f -> p t f", p=128), mean_t[:]
            )

    nc.compile()
    return nc


_CACHE = {}


def _get_compiled(plan, h, srcs, dsts, Ws, bs):
    cfg, in_maps = _prepare(plan, h, srcs, dsts, Ws, bs)
    key = (
        plan.N,
        plan.ncores,
        tuple(tuple(t) for t in cfg.T0s),
        tuple(tuple(t) for t in cfg.T1s),
    )
    if key not in _CACHE:
        _CACHE[key] = _build_kernel(cfg)
    return _CACHE[key], cfg, in_maps


def run(h, srcs, dsts, Ws, bs, N=None, ncores=8, trace=False):
    N = h.shape[0] if N is None else N
    plan = _make_plan(N, ncores)
    nc, cfg, in_maps = _get_compiled(plan, h, srcs, dsts, Ws, bs)
    res = run_bass_kernel_spmd(
        nc, in_maps, core_ids=list(range(ncores)), trace=trace
    )
    out = np.concatenate(
        [res.results[c]["out"][: plan.npc] for c in range(ncores)], axis=0
    )
    return out[:N], res


def kernel(h, src0, dst0, src1, dst1, src2, dst2, W0, b0, W1, b1, W2, b2):
    h = np.asarray(h, np.float32)
    srcs = [np.asarray(s, np.int64) for s in (src0, src1, src2)]
    dsts = [np.asarray(d, np.int64) for d in (dst0, dst1, dst2)]
    Ws = [np.asarray(w, np.float32) for w in (W0, W1, W2)]
    bs = [np.asarray(b, np.float32) for b in (b0, b1, b2)]
    out, _ = run(h, srcs, dsts, Ws, bs)
    return out.astype(np.float32)


# revision 6
# speedup vs baseline: 1.1222x; 1.0019x over previous
"""HAN layer (3-metapath GCN mean) Trainium2 Bass kernel, 8-core SPMD.

Strategy (dst-sharded pull, h-gather variant):
  - GCN linearity: agg(h*rs_out) @ W == agg((h*rs_out) @ W).  So gather the
    host-prepared bf16 pre-scaled h tables (h * rsqrt(deg_out_m), 128 feats =
    256B rows, the dma_gather minimum elem) and apply W AFTER aggregation.
    This removes the on-device x-table phase entirely: gathers start at t~0
    and the SWDGE descriptor-generation wall (4 queues, ~2.25ns/desc) is the
    only remaining critical path.
  - nodes range-sharded across 8 cores (6250 each); per (core, metapath,
    chunk): in-edges of owned nodes laid out by the host into a gather slot
    schedule: nodes sorted by chunk-degree descending, blocks of 128 nodes,
    per-block fixed column count T[b] (elementwise max over the 8 cores so
    the program is identical across cores).  4096-idx dma_gather calls rotate
    over the 4 SWDGE queues; VectorE tensor_reduce sums each block's columns
    (bf16 in, fp32 out); raw block sums are dma_scatter_add-ed (un-permuting)
    into a zeroed per-metapath DRAM aggregate [npc_pad, 128].
  - final per-metapath pass: per 128-node block, PE-transpose agg, bf16
    matmul by W_m, then rsqrt(deg_in)/3 scale + bias/3 + relu (DVE+ACT),
    accumulated into the mean; one output DMA.  Host concatenates cores.
"""

import numpy as np
import ml_dtypes

import concourse.bass as bass
import concourse.tile as tile
from concourse import bacc, mybir
from concourse.bass_utils import run_bass_kernel_spmd
from concourse.masks import make_identity

F_IN, F_OUT, NMP = 128, 64, 3
GROUP_MAX_COLS = 32  # max slot columns per dma_gather call (4096 idxs)


def _wrap16(flat):
    """slot i -> (partition i%16, free i//16), replicated to 128 partitions."""
    a = flat.astype(np.int16).reshape(-1, 16).T.copy()  # [16, S/16]
    return np.tile(a, (8, 1))


class _NS:
    pass


def _make_plan(N, ncores):
    p = _NS()
    p.N, p.ncores = N, ncores
    p.npc = N // ncores
    p.NBP = (p.npc + 127) // 128
    p.npc_pad = p.NBP * 128
    p.NT = (N + 511) // 512 * 4
    p.N_pad = p.NT * 128
    p.tilesA = (p.NT + 1) // 2
    p.tilesB = p.NT - p.tilesA
    p.CHN = p.tilesA * 128
    p.rowsA = p.tilesA * 128 + 128
    p.rowsB = p.tilesB * 128 + 128
    p.zeroA = p.tilesA * 128
    p.zeroB = p.tilesB * 128
    assert p.rowsA < 32768 and p.rowsB < 32768
    return p


def _build_stream(plan, d_sel, li_sel, order, Ts, zero_base):
    TOT = int(Ts.sum())
    fill = (zero_base + (np.arange(TOT * 128) % 128)).astype(np.int16)
    if TOT == 0 or len(d_sel) == 0:
        return fill
    B = np.zeros(plan.NBP, np.int64)
    B[1:] = np.cumsum(Ts)[:-1]
    rank = np.empty(plan.npc, np.int64)
    rank[order] = np.arange(plan.npc)
    r_e = rank[d_sel]
    o = np.argsort(r_e, kind="stable")
    r_s = r_e[o]
    li = li_sel[o]
    starts = np.searchsorted(r_s, np.arange(plan.npc))
    k = np.arange(len(r_s)) - starts[r_s]
    blk = r_s // 128
    col = B[blk] + k
    assert (k < Ts[blk]).all()
    fill[col * 128 + (r_s % 128)] = li.astype(np.int16)
    return fill


def _groups(Ts):
    out, b, col, NB = [], 0, 0, len(Ts)
    while b < NB:
        if Ts[b] == 0:
            b += 1
            continue
        b_lo, col_lo, ncols = b, col, 0
        while b < NB and Ts[b] > 0 and (ncols == 0 or ncols + Ts[b] <= GROUP_MAX_COLS):
            ncols += int(Ts[b])
            col += int(Ts[b])
            b += 1
        out.append((b_lo, b, col_lo, ncols))
    return out


def _prepare(plan, h, srcs, dsts, Ws, bs):
    N, npc = plan.N, plan.npc

    rs_out, rs_in = [], []
    for m in range(NMP):
        do = np.clip(np.bincount(srcs[m], minlength=N), 1, None).astype(np.float64)
        di = np.clip(np.bincount(dsts[m], minlength=N), 1, None).astype(np.float64)
        rs_out.append((1.0 / np.sqrt(do)).astype(np.float32))
        rs_in.append((1.0 / np.sqrt(di)).astype(np.float32))

    # pre-scaled h chunk tables (bf16), trailing 128 zero rows = padding slots
    hA, hB = [], []
    for m in range(NMP):
        hs = h * rs_out[m][:, None]
        a = np.zeros((plan.rowsA, F_IN), np.float32)
        a[: plan.CHN] = hs[: plan.CHN]
        b_ = np.zeros((plan.rowsB, F_IN), np.float32)
        nb = N - plan.CHN
        b_[:nb] = hs[plan.CHN :]
        hA.append(a.astype(ml_dtypes.bfloat16))
        hB.append(b_.astype(ml_dtypes.bfloat16))

    wall = np.concatenate(Ws, axis=1).astype(ml_dtypes.bfloat16)
    ball3 = np.concatenate(
        [np.tile(b[None, :] / 3.0, (128, 1)) for b in bs], axis=1
    ).astype(np.float32)

    info = {}
    T0s = [np.zeros(plan.NBP, np.int64) for _ in range(NMP)]
    T1s = [np.zeros(plan.NBP, np.int64) for _ in range(NMP)]
    pad = plan.npc_pad - npc
    z = np.zeros(pad, np.int64)
    for c in range(plan.ncores):
        lo = c * npc
        for m in range(NMP):
            sel = (dsts[m] >= lo) & (dsts[m] < lo + npc)
            s = srcs[m][sel]
            d = dsts[m][sel] - lo
            in0 = s < plan.CHN
            c0 = np.bincount(d[in0], minlength=npc)
            c1 = np.bincount(d[~in0], minlength=npc)
            o0 = np.argsort(-c0, kind="stable")
            o1 = np.argsort(-c1, kind="stable")
            info[(c, m)] = (s, d, in0, o0, o1)
            T0s[m] = np.maximum(
                T0s[m], np.concatenate([c0[o0], z]).reshape(plan.NBP, 128).max(1)
            )
            T1s[m] = np.maximum(
                T1s[m], np.concatenate([c1[o1], z]).reshape(plan.NBP, 128).max(1)
            )

    cfg = _NS()
    cfg.plan = plan
    cfg.T0s, cfg.T1s = T0s, T1s
    cfg.g0 = [_groups(T0s[m]) for m in range(NMP)]
    cfg.g1 = [_groups(T1s[m]) for m in range(NMP)]
    cfg.TOT0 = [int(T0s[m].sum()) for m in range(NMP)]
    cfg.TOT1 = [int(T1s[m].sum()) for m in range(NMP)]

    in_maps = []
    for c in range(plan.ncores):
        im = {"wall": wall, "ball3": ball3}
        for m in range(NMP):
            im[f"hA{m}"] = hA[m]
            im[f"hB{m}"] = hB[m]
        lo = c * npc
        for m in range(NMP):
            s, d, in0, o0, o1 = info[(c, m)]
            liA = s[in0]
            liB = s[~in0] - plan.CHN
            st0 = _build_stream(plan, d[in0], liA, o0, T0s[m], plan.zeroA)
            st1 = _build_stream(plan, d[~in0], liB, o1, T1s[m], plan.zeroB)
            im[f"g0_{m}"] = (
                _wrap16(st0) if cfg.TOT0[m] else np.zeros((128, 8), np.int16)
            )
            im[f"g1_{m}"] = (
                _wrap16(st1) if cfg.TOT1[m] else np.zeros((128, 8), np.int16)
            )
            for ch, o in ((0, o0), (1, o1)):
                si = np.full(plan.npc_pad, -1, np.int64)
                si[:npc] = o
                im[f"si{ch}_{m}"] = _wrap16(si)
            # natural-order rsqrt(deg_in)/3 per (partition, block)
            full = np.zeros(plan.npc_pad, np.float32)
            full[:npc] = rs_in[m][lo : lo + npc] / 3.0
            im[f"ri_{m}"] = full.reshape(plan.NBP, 128).T.copy()
        in_maps.append(im)
    return cfg, in_maps


def _build_kernel(cfg):
    plan = cfg.plan
    nc = bacc.Bacc(
        "TRN2",
        target_bir_lowering=False,
        debug=False,
        num_devices=plan.ncores,
        num_swdge_queues=4,
    )
    dt = mybir.dt
    hA_d, hB_d = {}, {}
    for m in range(NMP):
        hA_d[m] = nc.dram_tensor(f"hA{m}", (plan.rowsA, F_IN), dt.bfloat16, kind="ExternalInput").ap()
        hB_d[m] = nc.dram_tensor(f"hB{m}", (plan.rowsB, F_IN), dt.bfloat16, kind="ExternalInput").ap()
    wall_d = nc.dram_tensor("wall", (128, F_OUT * NMP), dt.bfloat16, kind="ExternalInput").ap()
    ball3_d = nc.dram_tensor("ball3", (128, F_OUT * NMP), dt.float32, kind="ExternalInput").ap()
    g_d, si_d, ri_d, agg = {}, {}, {}, {}
    for m in range(NMP):
        w0 = max(cfg.TOT0[m] * 8, 8)
        w1 = max(cfg.TOT1[m] * 8, 8)
        g_d[(0, m)] = nc.dram_tensor(f"g0_{m}", (128, w0), dt.int16, kind="ExternalInput").ap()
        g_d[(1, m)] = nc.dram_tensor(f"g1_{m}", (128, w1), dt.int16, kind="ExternalInput").ap()
        si_d[(0, m)] = nc.dram_tensor(f"si0_{m}", (128, plan.npc_pad // 16), dt.int16, kind="ExternalInput").ap()
        si_d[(1, m)] = nc.dram_tensor(f"si1_{m}", (128, plan.npc_pad // 16), dt.int16, kind="ExternalInput").ap()
        ri_d[m] = nc.dram_tensor(f"ri_{m}", (128, plan.NBP), dt.float32, kind="ExternalInput").ap()
        agg[m] = nc.dram_tensor(f"agg_{m}", (plan.npc_pad, F_IN), dt.float32, kind="Internal").ap()
    out_d = nc.dram_tensor("out", (plan.npc_pad, F_OUT), dt.float32, kind="ExternalOutput").ap()

    with tile.TileContext(nc) as tc:
        with tc.tile_pool(name="const", bufs=1) as constp, \
             tc.tile_pool(name="ps", bufs=4, space="PSUM") as psp, \
             tc.tile_pool(name="psw", bufs=4, space="PSUM") as pswp, \
             tc.tile_pool(name="gst", bufs=3) as gstp, \
             tc.tile_pool(name="gath", bufs=8) as gp, \
             tc.tile_pool(name="accs", bufs=2) as accp, \
             tc.tile_pool(name="fin", bufs=4) as finp, \
             tc.tile_pool(name="agg_ld", bufs=3) as aglp, \
             tc.tile_pool(name="aggT", bufs=4) as agtp, \
             tc.tile_pool(name="mean", bufs=1) as meanp:

            # idx mega-stream preload for the first (m=0, chunk A) stream so
            # gathers start immediately.
            gst_t = {}

            def load_stream(ch, m):
                tot = cfg.TOT0[m] if ch == 0 else cfg.TOT1[m]
                w = max(tot * 8, 8)
                t = gstp.tile([128, w], dt.int16, name="gst")
                nc.sync.dma_start(t[:], g_d[(ch, m)][:, :w])
                gst_t[(ch, m)] = t

            load_stream(0, 0)

            wall_t = constp.tile([128, F_OUT * NMP], dt.bfloat16)
            nc.sync.dma_start(wall_t[:], wall_d[:])
            ball3_t = constp.tile([128, F_OUT * NMP], dt.float32)
            nc.sync.dma_start(ball3_t[:], ball3_d[:])
            ident = constp.tile([128, 128], dt.float32)
            make_identity(nc, ident[:])
            ri_t, si_t = {}, {}
            for m in range(NMP):
                ri_t[m] = constp.tile([128, plan.NBP], dt.float32, name=f"ri_t{m}")
                nc.sync.dma_start(ri_t[m][:], ri_d[m][:])
                for ch in (0, 1):
                    si_t[(ch, m)] = constp.tile(
                        [128, plan.npc_pad // 16], dt.int16, name=f"si_t{ch}_{m}"
                    )
                    nc.sync.dma_start(si_t[(ch, m)][:], si_d[(ch, m)][:])

            # zero the DRAM aggregates (scatter_add accumulates into them);
            # borrow an acc-pool buffer so no extra SBUF is held.
            zt = accp.tile([128, plan.NBP, F_IN], dt.float32, name="acc")
            nc.vector.memset(zt[:], 0.0)
            for m in range(NMP):
                nc.sync.dma_start(
                    agg[m][:].rearrange("n f -> (n f)").rearrange(
                        "(p x) -> p x", p=128
                    ),
                    zt[:].rearrange("p t f -> p (t f)"),
                )
            mean_t = meanp.tile([128, plan.NBP, F_OUT], dt.float32)
            nc.vector.memset(mean_t[:], 0.0)

            qrr = 0

            def do_scatter(ch, m, acc):
                # split across the 4 SWDGE queues (block-aligned pieces) so
                # the per-queue DGE walls shrink 4x, cutting the tail stall.
                nonlocal qrr
                NBP = plan.NBP
                bounds = [0, (NBP + 3) // 4, (NBP + 1) // 2, (3 * NBP) // 4, NBP]
                for pi in range(4):
                    b0, b1 = bounds[pi], bounds[pi + 1]
                    if b1 <= b0:
                        continue
                    s0, s1 = b0 * 128, b1 * 128
                    valid = max(0, min(plan.npc, s1) - s0)
                    if valid == 0:
                        continue
                    nc.gpsimd.dma_scatter_add(
                        out_ap=agg[m][:],
                        in_ap=acc[:, b0:b1, :],
                        idxs_ap=si_t[(ch, m)][:, b0 * 8 : b1 * 8],
                        num_idxs=s1 - s0,
                        num_idxs_reg=valid,
                        elem_size=F_IN,
                        single_packet=False,
                        queue_num=qrr,
                    )
                    qrr = (qrr + 1) % 4

            # finalize metapath m (called deferred, after both scatters):
            # e_m = relu((agg @ W_m)*ri/3 + b/3); mean += e_m
            def finalize(m):
                for b0 in range(0, plan.NBP, 4):
                    nb4 = min(4, plan.NBP - b0)
                    ag = aglp.tile([128, 4, F_IN], dt.float32, name="ag")
                    nc.sync.dma_start(
                        ag[:, :nb4, :],
                        agg[m][b0 * 128 : (b0 + nb4) * 128, :].rearrange(
                            "(t p) f -> p t f", p=128
                        ),
                    )
                    for b in range(b0, b0 + nb4):
                        _fin_block(m, b, ag[:, b - b0, :])

            def _fin_block(m, b, ag):
                    psT = psp.tile([128, 128], dt.float32, space="PSUM")
                    nc.tensor.transpose(out=psT[:], in_=ag[:], identity=ident[:])
                    agT = agtp.tile([128, 128], dt.bfloat16, name="agT")
                    nc.scalar.activation(
                        agT[:], psT[:], mybir.ActivationFunctionType.Copy
                    )
                    ps2 = pswp.tile([128, F_OUT], dt.float32, space="PSUM")
                    nc.tensor.matmul(
                        ps2[:],
                        lhsT=agT[:],
                        rhs=wall_t[:, m * F_OUT : (m + 1) * F_OUT],
                        start=True,
                        stop=True,
                    )
                    fin = finp.tile([128, F_OUT], dt.float32, name="fin")
                    nc.vector.scalar_tensor_tensor(
                        fin[:],
                        ps2[:],
                        ri_t[m][:, b : b + 1],
                        ball3_t[:, m * F_OUT : (m + 1) * F_OUT],
                        mybir.AluOpType.mult,
                        mybir.AluOpType.add,
                    )
                    nc.scalar.activation(
                        fin[:], fin[:], mybir.ActivationFunctionType.Relu
                    )
                    nc.vector.tensor_tensor(
                        mean_t[:, b, :], mean_t[:, b, :], fin[:],
                        mybir.AluOpType.add,
                    )
            nc.sync.dma_start(
                out_d[:].rearrange("(t p) f -> p t f", p=128), mean_t[:]
            )

            order = []
            for m in range(NMP):
                order.append((0, m))
                order.append((1, m))
            pending = []
            for oi, (ch, m) in enumerate(order):
                groups = cfg.g0[m] if ch == 0 else cfg.g1[m]
                Ts = cfg.T0s[m] if ch == 0 else cfg.T1s[m]
                tab = hA_d[m] if ch == 0 else hB_d[m]
                if oi + 1 < len(order):
                    load_stream(*order[oi + 1])
                acc = accp.tile([128, plan.NBP, F_IN], dt.float32, name="acc")
                nc.vector.memset(acc[:], 0.0)
                Bcols = np.zeros(plan.NBP, np.int64)
                Bcols[1:] = np.cumsum(Ts)[:-1]
                for gi, (b_lo, b_hi, col_lo, ncols) in enumerate(groups):
                    if gi == 6 and pending:
                        # previous chunk's scatter/finalize issue here so its
                        # vector-sem wait sits behind 6 queued gathers instead
                        # of stalling the gpsimd engine head-of-line.
                        for f_ in pending:
                            f_()
                        pending = []
                    it = gst_t[(ch, m)][:, col_lo * 8 : (col_lo + ncols) * 8]
                    gt = gp.tile([128, ncols, F_IN], dt.bfloat16, name="gt")
                    nc.gpsimd.dma_gather(
                        out_ap=gt[:],
                        in_ap=tab[:],
                        idxs_ap=it,
                        num_idxs=ncols * 128,
                        num_idxs_reg=ncols * 128,
                        elem_size=F_IN,
                        single_packet=False,
                        queue_num=qrr,
                    )
                    qrr = (qrr + 1) % 4
                    for b in range(b_lo, b_hi):
                        cl = int(Bcols[b] - col_lo)
                        w = int(Ts[b])
                        if w == 0:
                            continue
                        # pairwise-fold wide blocks with contiguous bf16 adds
                        # (2 elem/cyc) before the strided reduce (~3.7 cyc/elem)
                        while w >= 4:
                            if w % 2:
                                nc.vector.tensor_tensor(
                                    gt[:, cl, :], gt[:, cl, :],
                                    gt[:, cl + w - 1, :], mybir.AluOpType.add,
                                )
                                w -= 1
                            h = w // 2
                            nc.vector.tensor_tensor(
                                gt[:, cl : cl + h, :],
                                gt[:, cl : cl + h, :],
                                gt[:, cl + h : cl + 2 * h, :],
                                mybir.AluOpType.add,
                            )
                            w = h
                        view = gt[:, cl : cl + w, :].rearrange("p t f -> p f t")
                        nc.vector.tensor_reduce(
                            acc[:, b, :], view, mybir.AxisListType.X,
                            mybir.AluOpType.add,
                        )
                for f_ in pending:
                    f_()
                pending = []
                pending.append(
                    (lambda ch=ch, m=m, acc=acc: do_scatter(ch, m, acc))
                )
                if ch == 1:
                    pending.append(lambda m=m: finalize(m))
            for f_ in pending:
                f_()

            nc.sync.dma_start(
                out_d[:].rearrange("(t p) f -> p t f", p=128), mean_t[:]
            )

    nc.compile()
    return nc


_CACHE = {}


def _get_compiled(plan, h, srcs, dsts, Ws, bs):
    cfg, in_maps = _prepare(plan, h, srcs, dsts, Ws, bs)
    key = (
        plan.N,
        plan.ncores,
        tuple(tuple(t) for t in cfg.T0s),
        tuple(tuple(t) for t in cfg.T1s),
    )
    if key not in _CACHE:
        _CACHE[key] = _build_kernel(cfg)
    return _CACHE[key], cfg, in_maps


def run(h, srcs, dsts, Ws, bs, N=None, ncores=8, trace=False):
    N = h.shape[0] if N is None else N
    plan = _make_plan(N, ncores)
    nc, cfg, in_maps = _get_compiled(plan, h, srcs, dsts, Ws, bs)
    res = run_bass_kernel_spmd(
        nc, in_maps, core_ids=list(range(ncores)), trace=trace
    )
    out = np.concatenate(
        [res.results[c]["out"][: plan.npc] for c in range(ncores)], axis=0
    )
    return out[:N], res


def kernel(h, src0, dst0, src1, dst1, src2, dst2, W0, b0, W1, b1, W2, b2):
    h = np.asarray(h, np.float32)
    srcs = [np.asarray(s, np.int64) for s in (src0, src1, src2)]
    dsts = [np.asarray(d, np.int64) for d in (dst0, dst1, dst2)]
    Ws = [np.asarray(w, np.float32) for w in (W0, W1, W2)]
    bs = [np.asarray(b, np.float32) for b in (b0, b1, b2)]
    out, _ = run(h, srcs, dsts, Ws, bs)
    return out.astype(np.float32)


# revision 7
# speedup vs baseline: 1.1484x; 1.0233x over previous
"""HAN layer (3-metapath GCN mean) Trainium2 Bass kernel, 8-core SPMD.

Strategy (dst-sharded pull, h-gather variant):
  - GCN linearity: agg(h*rs_out) @ W == agg((h*rs_out) @ W).  So gather the
    host-prepared bf16 pre-scaled h tables (h * rsqrt(deg_out_m), 128 feats =
    256B rows, the dma_gather minimum elem) and apply W AFTER aggregation.
    This removes the on-device x-table phase entirely: gathers start at t~0
    and the SWDGE descriptor-generation wall (4 queues, ~2.25ns/desc) is the
    only remaining critical path.
  - nodes range-sharded across 8 cores (6250 each); per (core, metapath,
    chunk): in-edges of owned nodes laid out by the host into a gather slot
    schedule: nodes sorted by chunk-degree descending, blocks of 128 nodes,
    per-block fixed column count T[b] (elementwise max over the 8 cores so
    the program is identical across cores).  4096-idx dma_gather calls rotate
    over the 4 SWDGE queues; VectorE tensor_reduce sums each block's columns
    (bf16 in, fp32 out); raw block sums are dma_scatter_add-ed (un-permuting)
    into a zeroed per-metapath DRAM aggregate [npc_pad, 128].
  - final per-metapath pass: per 128-node block, PE-transpose agg, bf16
    matmul by W_m, then rsqrt(deg_in)/3 scale + bias/3 + relu (DVE+ACT),
    accumulated into the mean; one output DMA.  Host concatenates cores.
"""

import numpy as np
import ml_dtypes

import concourse.bass as bass
import concourse.tile as tile
from concourse import bacc, mybir
from concourse.bass_utils import run_bass_kernel_spmd
from concourse.masks import make_identity

F_IN, F_OUT, NMP = 128, 64, 3
GROUP_MAX_COLS = 32  # max slot columns per dma_gather call (4096 idxs)


def _wrap16(flat):
    """slot i -> (partition i%16, free i//16), replicated to 128 partitions."""
    a = flat.astype(np.int16).reshape(-1, 16).T.copy()  # [16, S/16]
    return np.tile(a, (8, 1))


class _NS:
    pass


def _make_plan(N, ncores):
    p = _NS()
    p.N, p.ncores = N, ncores
    p.npc = N // ncores
    p.NBP = (p.npc + 127) // 128
    p.npc_pad = p.NBP * 128
    p.NT = (N + 511) // 512 * 4
    p.N_pad = p.NT * 128
    p.tilesA = (p.NT + 1) // 2
    p.tilesB = p.NT - p.tilesA
    p.CHN = p.tilesA * 128
    p.rowsA = p.tilesA * 128 + 128
    p.rowsB = p.tilesB * 128 + 128
    p.zeroA = p.tilesA * 128
    p.zeroB = p.tilesB * 128
    assert p.rowsA < 32768 and p.rowsB < 32768
    return p


def _build_stream(plan, d_sel, li_sel, order, Ts, zero_base):
    TOT = int(Ts.sum())
    fill = (zero_base + (np.arange(TOT * 128) % 128)).astype(np.int16)
    if TOT == 0 or len(d_sel) == 0:
        return fill
    B = np.zeros(plan.NBP, np.int64)
    B[1:] = np.cumsum(Ts)[:-1]
    rank = np.empty(plan.npc, np.int64)
    rank[order] = np.arange(plan.npc)
    r_e = rank[d_sel]
    o = np.argsort(r_e, kind="stable")
    r_s = r_e[o]
    li = li_sel[o]
    starts = np.searchsorted(r_s, np.arange(plan.npc))
    k = np.arange(len(r_s)) - starts[r_s]
    blk = r_s // 128
    col = B[blk] + k
    assert (k < Ts[blk]).all()
    fill[col * 128 + (r_s % 128)] = li.astype(np.int16)
    return fill


def _groups(Ts):
    out, b, col, NB = [], 0, 0, len(Ts)
    while b < NB:
        if Ts[b] == 0:
            b += 1
            continue
        b_lo, col_lo, ncols = b, col, 0
        while b < NB and Ts[b] > 0 and (ncols == 0 or ncols + Ts[b] <= GROUP_MAX_COLS):
            ncols += int(Ts[b])
            col += int(Ts[b])
            b += 1
        out.append((b_lo, b, col_lo, ncols))
    return out


def _prepare(plan, h, srcs, dsts, Ws, bs):
    N, npc = plan.N, plan.npc

    rs_out, rs_in = [], []
    for m in range(NMP):
        do = np.clip(np.bincount(srcs[m], minlength=N), 1, None).astype(np.float64)
        di = np.clip(np.bincount(dsts[m], minlength=N), 1, None).astype(np.float64)
        rs_out.append((1.0 / np.sqrt(do)).astype(np.float32))
        rs_in.append((1.0 / np.sqrt(di)).astype(np.float32))

    # pre-scaled h chunk tables (bf16), trailing 128 zero rows = padding slots
    hA, hB = [], []
    for m in range(NMP):
        hs = h * rs_out[m][:, None]
        a = np.zeros((plan.rowsA, F_IN), np.float32)
        a[: plan.CHN] = hs[: plan.CHN]
        b_ = np.zeros((plan.rowsB, F_IN), np.float32)
        nb = N - plan.CHN
        b_[:nb] = hs[plan.CHN :]
        hA.append(a.astype(ml_dtypes.bfloat16))
        hB.append(b_.astype(ml_dtypes.bfloat16))

    wall = np.concatenate(Ws, axis=1).astype(ml_dtypes.bfloat16)
    ball3 = np.concatenate(
        [np.tile(b[None, :] / 3.0, (128, 1)) for b in bs], axis=1
    ).astype(np.float32)

    info = {}
    T0s = [np.zeros(plan.NBP, np.int64) for _ in range(NMP)]
    T1s = [np.zeros(plan.NBP, np.int64) for _ in range(NMP)]
    pad = plan.npc_pad - npc
    z = np.zeros(pad, np.int64)
    for c in range(plan.ncores):
        lo = c * npc
        for m in range(NMP):
            sel = (dsts[m] >= lo) & (dsts[m] < lo + npc)
            s = srcs[m][sel]
            d = dsts[m][sel] - lo
            in0 = s < plan.CHN
            c0 = np.bincount(d[in0], minlength=npc)
            c1 = np.bincount(d[~in0], minlength=npc)
            o0 = np.argsort(-c0, kind="stable")
            o1 = np.argsort(-c1, kind="stable")
            info[(c, m)] = (s, d, in0, o0, o1)
            T0s[m] = np.maximum(
                T0s[m], np.concatenate([c0[o0], z]).reshape(plan.NBP, 128).max(1)
            )
            T1s[m] = np.maximum(
                T1s[m], np.concatenate([c1[o1], z]).reshape(plan.NBP, 128).max(1)
            )

    cfg = _NS()
    cfg.plan = plan
    cfg.T0s, cfg.T1s = T0s, T1s
    cfg.g0 = [_groups(T0s[m]) for m in range(NMP)]
    cfg.g1 = [_groups(T1s[m]) for m in range(NMP)]
    cfg.TOT0 = [int(T0s[m].sum()) for m in range(NMP)]
    cfg.TOT1 = [int(T1s[m].sum()) for m in range(NMP)]

    in_maps = []
    for c in range(plan.ncores):
        im = {"wall": wall, "ball3": ball3}
        for m in range(NMP):
            im[f"hA{m}"] = hA[m]
            im[f"hB{m}"] = hB[m]
        lo = c * npc
        for m in range(NMP):
            s, d, in0, o0, o1 = info[(c, m)]
            liA = s[in0]
            liB = s[~in0] - plan.CHN
            st0 = _build_stream(plan, d[in0], liA, o0, T0s[m], plan.zeroA)
            st1 = _build_stream(plan, d[~in0], liB, o1, T1s[m], plan.zeroB)
            im[f"g0_{m}"] = (
                _wrap16(st0) if cfg.TOT0[m] else np.zeros((128, 8), np.int16)
            )
            im[f"g1_{m}"] = (
                _wrap16(st1) if cfg.TOT1[m] else np.zeros((128, 8), np.int16)
            )
            for ch, o in ((0, o0), (1, o1)):
                si = np.full(plan.npc_pad, -1, np.int64)
                si[:npc] = o
                im[f"si{ch}_{m}"] = _wrap16(si)
            # natural-order rsqrt(deg_in)/3 per (partition, block)
            full = np.zeros(plan.npc_pad, np.float32)
            full[:npc] = rs_in[m][lo : lo + npc] / 3.0
            im[f"ri_{m}"] = full.reshape(plan.NBP, 128).T.copy()
        in_maps.append(im)
    return cfg, in_maps


def _build_kernel(cfg):
    plan = cfg.plan
    nc = bacc.Bacc(
        "TRN2",
        target_bir_lowering=False,
        debug=False,
        num_devices=plan.ncores,
        num_swdge_queues=4,
    )
    dt = mybir.dt
    hA_d, hB_d = {}, {}
    for m in range(NMP):
        hA_d[m] = nc.dram_tensor(f"hA{m}", (plan.rowsA, F_IN), dt.bfloat16, kind="ExternalInput").ap()
        hB_d[m] = nc.dram_tensor(f"hB{m}", (plan.rowsB, F_IN), dt.bfloat16, kind="ExternalInput").ap()
    wall_d = nc.dram_tensor("wall", (128, F_OUT * NMP), dt.bfloat16, kind="ExternalInput").ap()
    ball3_d = nc.dram_tensor("ball3", (128, F_OUT * NMP), dt.float32, kind="ExternalInput").ap()
    g_d, si_d, ri_d, agg = {}, {}, {}, {}
    for m in range(NMP):
        w0 = max(cfg.TOT0[m] * 8, 8)
        w1 = max(cfg.TOT1[m] * 8, 8)
        g_d[(0, m)] = nc.dram_tensor(f"g0_{m}", (128, w0), dt.int16, kind="ExternalInput").ap()
        g_d[(1, m)] = nc.dram_tensor(f"g1_{m}", (128, w1), dt.int16, kind="ExternalInput").ap()
        si_d[(0, m)] = nc.dram_tensor(f"si0_{m}", (128, plan.npc_pad // 16), dt.int16, kind="ExternalInput").ap()
        si_d[(1, m)] = nc.dram_tensor(f"si1_{m}", (128, plan.npc_pad // 16), dt.int16, kind="ExternalInput").ap()
        ri_d[m] = nc.dram_tensor(f"ri_{m}", (128, plan.NBP), dt.float32, kind="ExternalInput").ap()
        agg[m] = nc.dram_tensor(f"agg_{m}", (plan.npc_pad, F_IN), dt.float32, kind="Internal").ap()
    out_d = nc.dram_tensor("out", (plan.npc_pad, F_OUT), dt.float32, kind="ExternalOutput").ap()

    with tile.TileContext(nc) as tc:
        with tc.tile_pool(name="const", bufs=1) as constp, \
             tc.tile_pool(name="ps", bufs=4, space="PSUM") as psp, \
             tc.tile_pool(name="psw", bufs=4, space="PSUM") as pswp, \
             tc.tile_pool(name="gst", bufs=3) as gstp, \
             tc.tile_pool(name="gath", bufs=9) as gp, \
             tc.tile_pool(name="accs", bufs=2) as accp, \
             tc.tile_pool(name="fin", bufs=4) as finp, \
             tc.tile_pool(name="agg_ld", bufs=3) as aglp, \
             tc.tile_pool(name="aggT", bufs=4) as agtp, \
             tc.tile_pool(name="mean", bufs=1) as meanp:

            # idx mega-stream preload for the first (m=0, chunk A) stream so
            # gathers start immediately.
            gst_t = {}

            def load_stream(ch, m):
                tot = cfg.TOT0[m] if ch == 0 else cfg.TOT1[m]
                w = max(tot * 8, 8)
                t = gstp.tile([128, w], dt.int16, name="gst")
                nc.sync.dma_start(t[:], g_d[(ch, m)][:, :w])
                gst_t[(ch, m)] = t

            load_stream(0, 0)

            wall_t = constp.tile([128, F_OUT * NMP], dt.bfloat16)
            nc.sync.dma_start(wall_t[:], wall_d[:])
            ball3_t = constp.tile([128, F_OUT * NMP], dt.float32)
            nc.sync.dma_start(ball3_t[:], ball3_d[:])
            ident = constp.tile([128, 128], dt.float32)
            make_identity(nc, ident[:])
            ri_t, si_t = {}, {}
            for m in range(NMP):
                ri_t[m] = constp.tile([128, plan.NBP], dt.float32, name=f"ri_t{m}")
                nc.sync.dma_start(ri_t[m][:], ri_d[m][:])
                for ch in (0, 1):
                    si_t[(ch, m)] = constp.tile(
                        [128, plan.npc_pad // 16], dt.int16, name=f"si_t{ch}_{m}"
                    )
                    nc.sync.dma_start(si_t[(ch, m)][:], si_d[(ch, m)][:])

            # zero the DRAM aggregates (scatter_add accumulates into them);
            # borrow an acc-pool buffer so no extra SBUF is held.
            zt = accp.tile([128, plan.NBP, F_IN], dt.float32, name="acc")
            nc.vector.memset(zt[:], 0.0)
            for m in range(NMP):
                nc.sync.dma_start(
                    agg[m][:].rearrange("n f -> (n f)").rearrange(
                        "(p x) -> p x", p=128
                    ),
                    zt[:].rearrange("p t f -> p (t f)"),
                )
            mean_t = meanp.tile([128, plan.NBP, F_OUT], dt.float32)
            nc.vector.memset(mean_t[:], 0.0)

            qrr = 0

            def do_scatter(ch, m, acc):
                # split across the 4 SWDGE queues (block-aligned pieces) so
                # the per-queue DGE walls shrink 4x, cutting the tail stall.
                nonlocal qrr
                NBP = plan.NBP
                bounds = [0, (NBP + 3) // 4, (NBP + 1) // 2, (3 * NBP) // 4, NBP]
                for pi in range(4):
                    b0, b1 = bounds[pi], bounds[pi + 1]
                    if b1 <= b0:
                        continue
                    s0, s1 = b0 * 128, b1 * 128
                    valid = max(0, min(plan.npc, s1) - s0)
                    if valid == 0:
                        continue
                    nc.gpsimd.dma_scatter_add(
                        out_ap=agg[m][:],
                        in_ap=acc[:, b0:b1, :],
                        idxs_ap=si_t[(ch, m)][:, b0 * 8 : b1 * 8],
                        num_idxs=s1 - s0,
                        num_idxs_reg=valid,
                        elem_size=F_IN,
                        single_packet=False,
                        queue_num=qrr,
                    )
                    qrr = (qrr + 1) % 4

            # finalize metapath m (called deferred, after both scatters):
            # e_m = relu((agg @ W_m)*ri/3 + b/3); mean += e_m
            def finalize(m):
                for b0 in range(0, plan.NBP, 4):
                    nb4 = min(4, plan.NBP - b0)
                    ag = aglp.tile([128, 4, F_IN], dt.float32, name="ag")
                    nc.sync.dma_start(
                        ag[:, :nb4, :],
                        agg[m][b0 * 128 : (b0 + nb4) * 128, :].rearrange(
                            "(t p) f -> p t f", p=128
                        ),
                    )
                    for b in range(b0, b0 + nb4):
                        _fin_block(m, b, ag[:, b - b0, :])

            def _fin_block(m, b, ag):
                    psT = psp.tile([128, 128], dt.float32, space="PSUM")
                    nc.tensor.transpose(out=psT[:], in_=ag[:], identity=ident[:])
                    agT = agtp.tile([128, 128], dt.bfloat16, name="agT")
                    nc.scalar.activation(
                        agT[:], psT[:], mybir.ActivationFunctionType.Copy
                    )
                    ps2 = pswp.tile([128, F_OUT], dt.float32, space="PSUM")
                    nc.tensor.matmul(
                        ps2[:],
                        lhsT=agT[:],
                        rhs=wall_t[:, m * F_OUT : (m + 1) * F_OUT],
                        start=True,
                        stop=True,
                    )
                    fin = finp.tile([128, F_OUT], dt.float32, name="fin")
                    nc.vector.scalar_tensor_tensor(
                        fin[:],
                        ps2[:],
                        ri_t[m][:, b : b + 1],
                        ball3_t[:, m * F_OUT : (m + 1) * F_OUT],
                        mybir.AluOpType.mult,
                        mybir.AluOpType.add,
                    )
                    nc.scalar.activation(
                        fin[:], fin[:], mybir.ActivationFunctionType.Relu
                    )
                    nc.vector.tensor_tensor(
                        mean_t[:, b, :], mean_t[:, b, :], fin[:],
                        mybir.AluOpType.add,
                    )
            nc.sync.dma_start(
                out_d[:].rearrange("(t p) f -> p t f", p=128), mean_t[:]
            )

            order = []
            for m in range(NMP):
                order.append((0, m))
                order.append((1, m))
            pending = []
            for oi, (ch, m) in enumerate(order):
                groups = cfg.g0[m] if ch == 0 else cfg.g1[m]
                Ts = cfg.T0s[m] if ch == 0 else cfg.T1s[m]
                tab = hA_d[m] if ch == 0 else hB_d[m]
                if oi + 1 < len(order):
                    load_stream(*order[oi + 1])
                acc = accp.tile([128, plan.NBP, F_IN], dt.float32, name="acc")
                nc.vector.memset(acc[:], 0.0)
                Bcols = np.zeros(plan.NBP, np.int64)
                Bcols[1:] = np.cumsum(Ts)[:-1]
                for gi, (b_lo, b_hi, col_lo, ncols) in enumerate(groups):
                    if gi == 6 and pending:
                        # previous chunk's scatter/finalize issue here so its
                        # vector-sem wait sits behind 6 queued gathers instead
                        # of stalling the gpsimd engine head-of-line.
                        for f_ in pending:
                            f_()
                        pending = []
                    it = gst_t[(ch, m)][:, col_lo * 8 : (col_lo + ncols) * 8]
                    gt = gp.tile([128, ncols, F_IN], dt.bfloat16, name="gt")
                    nc.gpsimd.dma_gather(
                        out_ap=gt[:],
                        in_ap=tab[:],
                        idxs_ap=it,
                        num_idxs=ncols * 128,
                        num_idxs_reg=ncols * 128,
                        elem_size=F_IN,
                        single_packet=False,
                        queue_num=qrr,
                    )
                    qrr = (qrr + 1) % 4
                    for b in range(b_lo, b_hi):
                        cl = int(Bcols[b] - col_lo)
                        w = int(Ts[b])
                        if w == 0:
                            continue
                        # pairwise-fold wide blocks with contiguous bf16 adds
                        # (2 elem/cyc) before the strided reduce (~3.7 cyc/elem)
                        while w >= 4:
                            if w % 2:
                                nc.vector.tensor_tensor(
                                    gt[:, cl, :], gt[:, cl, :],
                                    gt[:, cl + w - 1, :], mybir.AluOpType.add,
                                )
                                w -= 1
                            h = w // 2
                            nc.vector.tensor_tensor(
                                gt[:, cl : cl + h, :],
                                gt[:, cl : cl + h, :],
                                gt[:, cl + h : cl + 2 * h, :],
                                mybir.AluOpType.add,
                            )
                            w = h
                        view = gt[:, cl : cl + w, :].rearrange("p t f -> p f t")
                        nc.vector.tensor_reduce(
                            acc[:, b, :], view, mybir.AxisListType.X,
                            mybir.AluOpType.add,
                        )
                for f_ in pending:
                    f_()
                pending = []
                pending.append(
                    (lambda ch=ch, m=m, acc=acc: do_scatter(ch, m, acc))
                )
                if ch == 1:
                    pending.append(lambda m=m: finalize(m))
            for f_ in pending:
                f_()

            nc.sync.dma_start(
                out_d[:].rearrange("(t p) f -> p t f", p=128), mean_t[:]
            )

    nc.compile()
    return nc


_CACHE = {}


def _get_compiled(plan, h, srcs, dsts, Ws, bs):
    cfg, in_maps = _prepare(plan, h, srcs, dsts, Ws, bs)
    key = (
        plan.N,
        plan.ncores,
        tuple(tuple(t) for t in cfg.T0s),
        tuple(tuple(t) for t in cfg.T1s),
    )
    if key not in _CACHE:
        _CACHE[key] = _build_kernel(cfg)
    return _CACHE[key], cfg, in_maps


def run(h, srcs, dsts, Ws, bs, N=None, ncores=8, trace=False):
    N = h.shape[0] if N is None else N
    plan = _make_plan(N, ncores)
    nc, cfg, in_maps = _get_compiled(plan, h, srcs, dsts, Ws, bs)
    res = run_bass_kernel_spmd(
        nc, in_maps, core_ids=list(range(ncores)), trace=trace
    )
    out = np.concatenate(
        [res.results[c]["out"][: plan.npc] for c in range(ncores)], axis=0
    )
    return out[:N], res


def kernel(h, src0, dst0, src1, dst1, src2, dst2, W0, b0, W1, b1, W2, b2):
    h = np.asarray(h, np.float32)
    srcs = [np.asarray(s, np.int64) for s in (src0, src1, src2)]
    dsts = [np.asarray(d, np.int64) for d in (dst0, dst1, dst2)]
    Ws = [np.asarray(w, np.float32) for w in (W0, W1, W2)]
    bs = [np.asarray(b, np.float32) for b in (b0, b1, b2)]
    out, _ = run(h, srcs, dsts, Ws, bs)
    return out.astype(np.float32)
